# revision 46
# baseline (speedup 1.0000x reference)
"""DirGNN(GCN x2, both directions) + GATv2 + mean-pool + MLP head on 8 trn2 cores.

bf16 variant: tables, gathered rows, one-hot matrices and matmul operands in
bf16 (PSUM/aggregation stays f32). Halves gather/collective bytes and runs the
PE at 4x the fp32 matmul rate.

Sharding: nodes in 8 equal contiguous ranges (batch is sorted, so graphs are
contiguous node ranges). Per core:
  - edge aggregation via dma_gather (256B rows) + one-hot is_equal + PE matmul
    scatter into per-128-node-window PSUM, accumulated in SBUF.
  - tables (dinv-scaled h, gl) are produced locally and AllGather'd.
  - pooling partial sums AllReduce'd; head MLP computed redundantly per core.

Host preprocessing is index/structure only (sorting, bucketing, padding,
int->f32/bf16 encodes); all model FLOPs run on device.
"""
import numpy as np
import ml_dtypes
import concourse.bacc as bacc
import concourse.bass as bass
import concourse.mybir as mybir
import concourse.tile as tile

P = 128
NCORES = 8
D = 128          # feature dim == HC
H, C = 4, 32
CLS = 2
HID = 128
ALPHA = 0.5
NEG = 0.2
F32 = mybir.dt.float32
BF = mybir.dt.bfloat16
NPBF = ml_dtypes.bfloat16
I16 = mybir.dt.int16
AOT = mybir.AluOpType
AFT = mybir.ActivationFunctionType
AXX = mybir.AxisListType


class Cfg:
    def __init__(self, N, E, G, chunk=32768, call=1024):
        self.N, self.E, self.G = N, E, G
        assert N % NCORES == 0
        self.N0 = N // NCORES                       # real nodes per core
        self.PADN0 = ((self.N0 + P - 1) // P) * P   # padded nodes per core
        self.W = self.PADN0 // P                    # node windows per core
        self.TAB = NCORES * self.PADN0              # padded global table rows
        self.CHUNK = chunk
        self.NCH = (self.TAB + chunk - 1) // chunk
        assert chunk <= 32768
        self.GPAD = ((G + P - 1) // P) * P
        self.GW = self.GPAD // P
        self.CALL = call                            # gather idxs per call
        assert call % P == 0


# ---------------------------------------------------------------- host side

def _wrap_idx_call(idx):
    """int array [n] (n%16==0) -> int16 [128, n/16] wrapped + replicated."""
    n = idx.shape[0]
    w = idx.reshape(n // 16, 16).T.astype(np.int16)
    return np.tile(w, (8, 1))


def _pad_to(a, n, val):
    return np.concatenate([a, np.full(n - a.shape[0], val, a.dtype)])


class EdgePass:
    """Host-built, statically-shared layout of one edge pass for all cores."""

    def __init__(self, cfg, src_list, dst_list, ea_list=None):
        cf = cfg
        self.gat = ea_list is not None
        padded = lambda n: (n // cf.N0) * cf.PADN0 + (n % cf.N0)
        percore = []
        counts = np.zeros((NCORES, cf.NCH, cf.W), np.int64)
        for c in range(NCORES):
            src, dst = src_list[c], dst_list[c]
            dstloc = dst - c * cf.N0
            assert (dstloc >= 0).all() and (dstloc < cf.N0).all()
            gidx = padded(src)
            k = gidx // cf.CHUNK
            w = dstloc >> 7
            order = np.lexsort((dstloc, w, k))
            ea = ea_list[c][order] if self.gat else None
            percore.append((gidx[order], dstloc[order], k[order], w[order], ea))
            np.add.at(counts[c], (k[order], w[order]), 1)
        S = np.maximum.reduce([np.ceil(counts[c] / P).astype(np.int64) for c in range(NCORES)])
        self.S = S                                   # [NCH, W] subtiles per group
        self.Smax = int(S.max()) if S.size else 0
        self.L = int(S.sum()) * P                    # padded edges per core
        # call layout per chunk
        self.calls = []                              # per k: list of call sizes
        self.sub2call = []                           # per subtile: (call_id, col)
        call_id = 0
        for k in range(cf.NCH):
            Lk = int(S[k].sum()) * P
            rem = Lk
            sizes = []
            while rem > 0:
                n = min(cf.CALL, rem)
                sizes.append(n)
                for col in range(n // P):
                    self.sub2call.append((call_id, col))
                call_id += 1
                rem -= n
            self.calls.append(sizes)
        self.n_calls = call_id
        self.nsub = self.L // P

        self.idx16, self.dstrel, self.ea, self.gridx16 = [], [], [], []
        for c in range(NCORES):
            gidx, dstloc, kk, ww, ea = percore[c]
            gi_parts, dr_parts, ea_parts, gr_parts = [], [], [], []
            for k in range(cf.NCH):
                for w in range(cf.W):
                    n = int(S[k, w]) * P
                    if n == 0:
                        continue
                    m = (kk == k) & (ww == w)
                    gi_parts.append(_pad_to(gidx[m] - k * cf.CHUNK, n, 0))
                    dr_parts.append(_pad_to((dstloc[m] - w * P).astype(np.float32), n, -1.0))
                    if self.gat:
                        ea_parts.append(_pad_to(ea[m].astype(np.float32), n, 0.0))
                        gr_parts.append(_pad_to(dstloc[m], n, 0))
            gi = np.concatenate(gi_parts) if gi_parts else np.zeros(0, np.int64)
            dr = np.concatenate(dr_parts) if dr_parts else np.zeros(0, np.float32)
            assert gi.shape[0] == self.L
            assert (gi >= 0).all() and (gi < 32768).all()

            def wrap_all(arr):
                off, wraps = 0, []
                for k in range(cf.NCH):
                    for n in self.calls[k]:
                        wraps.append(_wrap_idx_call(arr[off:off + n]))
                        off += n
                return (np.concatenate(wraps, axis=1) if wraps
                        else np.zeros((P, 0), np.int16))

            self.idx16.append(wrap_all(gi))
            self.dstrel.append(np.ascontiguousarray(dr.reshape(-1, P).T).astype(NPBF))
            if self.gat:
                self.ea.append(np.ascontiguousarray(
                    np.concatenate(ea_parts).reshape(-1, P).T).astype(NPBF))
                self.gridx16.append(wrap_all(np.concatenate(gr_parts)))

        # groups: per (k,w) with S>0: list of runs (sub0, nsub, call_id, col0)
        self.groups = []
        sub = 0
        for k in range(cf.NCH):
            for w in range(cf.W):
                s = int(S[k, w])
                if s == 0:
                    continue
                runs, i = [], sub
                while i < sub + s:
                    cid, col = self.sub2call[i]
                    n = 1
                    while i + n < sub + s and self.sub2call[i + n][0] == cid:
                        n += 1
                    runs.append((i, n, cid, col))
                    i += n
                self.groups.append((k, w, runs))
                sub += s
        assert sub == self.nsub


def preprocess(inputs, cfg):
    cf = cfg
    x = np.asarray(inputs["x"], np.float32)
    ei = np.asarray(inputs["edge_index"])
    ea = np.asarray(inputs["edge_attr"], np.float32).reshape(-1)
    batch = np.asarray(inputs["batch"]).astype(np.int64)
    src, dst = ei[0].astype(np.int64), ei[1].astype(np.int64)
    N, G = cf.N, cf.G

    deg_in = np.bincount(dst, minlength=N).astype(np.float32)
    deg_out = np.bincount(src, minlength=N).astype(np.float32)
    cnt = np.bincount(batch, minlength=G).astype(np.float32)

    core_of = lambda n: n // cf.N0
    m_in = [core_of(dst) == c for c in range(NCORES)]
    m_out = [core_of(src) == c for c in range(NCORES)]
    passes = {
        "in": EdgePass(cf, [src[m] for m in m_in], [dst[m] for m in m_in]),
        "out": EdgePass(cf, [dst[m] for m in m_out], [src[m] for m in m_out]),
    }
    gsrc, gdst, gea = [], [], []
    for c in range(NCORES):
        loop = np.arange(c * cf.N0, (c + 1) * cf.N0, dtype=np.int64)
        gsrc.append(np.concatenate([src[m_in[c]], loop]))
        gdst.append(np.concatenate([dst[m_in[c]], loop]))
        gea.append(np.concatenate([ea[m_in[c]], np.ones(cf.N0, np.float32)]))
    passes["gat"] = EdgePass(cf, gsrc, gdst, gea)

    iota = np.tile(np.arange(P, dtype=np.float32)[None, :], (P, 1))
    in_maps = []
    for c in range(NCORES):
        lo, hi = c * cf.N0, (c + 1) * cf.N0
        xl = np.zeros((cf.PADN0, D), NPBF)
        xl[:cf.N0] = x[lo:hi].astype(NPBF)
        nodecol = lambda v, pad: np.ascontiguousarray(
            _pad_to(v[lo:hi].astype(np.float32), cf.PADN0, pad).reshape(cf.W, P).T)
        m = {
            "x_local": xl,
            "deg_in_f": nodecol(deg_in, 0.0),
            "deg_out_f": nodecol(deg_out, 0.0),
            "batchloc": nodecol(batch.astype(np.float32), -1.0).astype(NPBF),
            "cnt_f": np.ascontiguousarray(_pad_to(cnt, cf.GPAD, 0.0).reshape(cf.GW, P).T),
            "iota_in": iota.astype(NPBF),
            "dir_w_in": np.asarray(inputs["dir_w_in"], np.float32).reshape(2 * D, D),
            "dir_b_in": np.asarray(inputs["dir_b_in"], np.float32),
            "dir_w_out": np.asarray(inputs["dir_w_out"], np.float32).reshape(2 * D, D),
            "dir_b_out": np.asarray(inputs["dir_b_out"], np.float32),
            "gat_wl": np.asarray(inputs["gat_wl"], np.float32),
            "gat_wr": np.asarray(inputs["gat_wr"], np.float32),
            "gat_we": np.asarray(inputs["gat_we"], np.float32),
            "gat_att": np.asarray(inputs["gat_att"], np.float32).reshape(1, D),
            "gat_b": np.asarray(inputs["gat_b"], np.float32).reshape(D, 1),
            "w1": np.asarray(inputs["w1"], np.float32),
            "b1": np.asarray(inputs["b1"], np.float32).reshape(1, HID),
            "w2": np.asarray(inputs["w2"], np.float32),
            "b2": np.asarray(inputs["b2"], np.float32).reshape(1, CLS),
        }
        for pname, ep in passes.items():
            m[f"idx_{pname}"] = ep.idx16[c]
            m[f"dstrel_{pname}"] = ep.dstrel[c]
            if ep.gat:
                m[f"ea_{pname}"] = ep.ea[c]
                m[f"gridx_{pname}"] = ep.gridx16[c]
        in_maps.append(m)
    return in_maps, passes


# ---------------------------------------------------------------- device side

def build_program(cfg, passes, stop_after=None):
    stop_after = None
    cf = cfg
    nc = bacc.Bacc("TRN2", target_bir_lowering=False, debug=False, num_devices=NCORES,
                   num_swdge_queues=2)

    def inp(name, shape, dt=F32):
        return nc.dram_tensor(name, shape, dt, kind="ExternalInput")

    x_local = inp("x_local", [cf.PADN0, D], BF)
    deg_in_f = inp("deg_in_f", [P, cf.W])
    deg_out_f = inp("deg_out_f", [P, cf.W])
    batchloc_in = inp("batchloc", [P, cf.W], BF)
    cnt_f = inp("cnt_f", [P, cf.GW])
    iota_in = inp("iota_in", [P, P], BF)
    dir_w_in = inp("dir_w_in", [2 * D, D])
    dir_b_in = inp("dir_b_in", [2, D])
    dir_w_out = inp("dir_w_out", [2 * D, D])
    dir_b_out = inp("dir_b_out", [2, D])
    gat_wl = inp("gat_wl", [D, D])
    gat_wr = inp("gat_wr", [D, D])
    gat_we = inp("gat_we", [1, D])
    gat_att = inp("gat_att", [1, D])
    gat_b = inp("gat_b", [D, 1])
    w1_in = inp("w1", [2 * D, HID])
    b1_in = inp("b1", [1, HID])
    w2_in = inp("w2", [HID, CLS])
    b2_in = inp("b2", [1, CLS])
    pin = {}
    for pname, ep in passes.items():
        pin[pname] = {
            "idx": inp(f"idx_{pname}", [P, ep.L // 16], I16),
            "dstrel": inp(f"dstrel_{pname}", [P, ep.L // P], BF),
        }
        if ep.gat:
            pin[pname]["ea"] = inp(f"ea_{pname}", [P, ep.L // P], BF)
            pin[pname]["gridx"] = inp(f"gridx_{pname}", [P, ep.L // 16], I16)

    out_t = nc.dram_tensor("out", [cf.GPAD, CLS], F32, kind="ExternalOutput")

    loc = {n: nc.dram_tensor(f"{n}_loc", [cf.PADN0, D], BF, kind="Internal")
           for n in ("t1i", "t1o", "t2i", "t2o", "gl")}
    gr_loc = nc.dram_tensor("gr_loc", [cf.PADN0, D], BF, kind="Internal")
    tabs = {n: nc.dram_tensor(f"{n}_tab", [cf.TAB, D], BF, kind="Internal", addr_space="Shared")
            for n in ("t1i", "t1o", "t2i", "t2o", "gl")}
    pool_part = nc.dram_tensor("pool_part", [cf.GPAD, 2 * D], F32, kind="Internal")
    pool_red = nc.dram_tensor("pool_red", [cf.GPAD, 2 * D], F32, kind="Internal", addr_space="Shared")
    RG = [list(range(NCORES))]
    Smax = max(ep.Smax for ep in passes.values())
    RUNMAX = Smax

    with tile.TileContext(nc) as tc:
        with (
            tc.tile_pool(name="const", bufs=1) as cpool,
            tc.tile_pool(name="agg", bufs=1) as aggpool,
            tc.tile_pool(name="work", bufs=3) as pool,
            tc.tile_pool(name="work2", bufs=2) as pool2,
            tc.tile_pool(name="psum", bufs=2, space="PSUM") as psum,
        ):
            # ---- constants / weights
            iota = cpool.tile([P, P], BF)
            nc.scalar.dma_start(iota[:], iota_in[:, :])
            iota3 = iota[:].rearrange("p (o q) -> p o q", o=1)
            ones_row = cpool.tile([1, P], BF)
            nc.vector.memset(ones_row[:], 1.0)
            from concourse.masks import make_identity
            ident = cpool.tile([P, P], BF)
            make_identity(nc, ident[:])

            def pe_transpose(src_ap, out_tile_ap):
                tp = psum.tile([P, P], BF, space="PSUM", tag="tpsum", name="tp")
                nc.tensor.transpose(out=tp[:, :], in_=src_ap, identity=ident[:])
                nc.vector.tensor_copy(out=out_tile_ap, in_=tp[:, :])

            def load_w(src_ap, shape, name, dt=F32):
                t = cpool.tile(shape, dt, name=name)
                nc.scalar.dma_start(t[:], src_ap)
                return t

            def to_bf(t, shape, name):
                b = cpool.tile(shape, BF, name=name)
                nc.vector.tensor_copy(out=b[:], in_=t[:])
                return b

            Wi_f = [load_w(dir_w_in[i * D:(i + 1) * D, :], [D, D], f"cwif{i}") for i in range(2)]
            Wo_f = [load_w(dir_w_out[i * D:(i + 1) * D, :], [D, D], f"cwof{i}") for i in range(2)]
            for i in range(2):
                nc.vector.tensor_scalar_mul(out=Wi_f[i][:], in0=Wi_f[i][:], scalar1=1.0 - ALPHA)
                nc.vector.tensor_scalar_mul(out=Wo_f[i][:], in0=Wo_f[i][:], scalar1=ALPHA)
            Wi = [to_bf(Wi_f[i], [D, D], f"cwi{i}") for i in range(2)]
            Wo = [to_bf(Wo_f[i], [D, D], f"cwo{i}") for i in range(2)]
            bcomb = []
            for i in range(2):
                bi = load_w(dir_b_in[i:i + 1, :], [1, D], f"cbif{i}")
                bo = load_w(dir_b_out[i:i + 1, :], [1, D], f"cbof{i}")
                nc.vector.tensor_scalar_mul(out=bi[:], in0=bi[:], scalar1=1.0 - ALPHA)
                nc.vector.tensor_scalar_mul(out=bo[:], in0=bo[:], scalar1=ALPHA)
                nc.vector.tensor_tensor(out=bi[:], in0=bi[:], in1=bo[:], op=AOT.add)
                bcomb.append(to_bf(bi, [1, D], f"cbi{i}"))
            Wl = to_bf(load_w(gat_wl[:, :], [D, D], "cwlf"), [D, D], "cwl")
            Wr = to_bf(load_w(gat_wr[:, :], [D, D], "cwrf"), [D, D], "cwr")
            w1t = [to_bf(load_w(w1_in[i * D:(i + 1) * D, :], [D, HID], f"cw1f{i}"), [D, HID], f"cw1{i}")
                   for i in range(2)]
            b1r = load_w(b1_in[:, :], [1, HID], "cb1r")
            w2t = to_bf(load_w(w2_in[:, :], [HID, CLS], "cw2f"), [HID, CLS], "cw2t")
            b2r = to_bf(load_w(b2_in[:, :], [1, CLS], "cb2f"), [1, CLS], "cb2r")
            we_rep_f = cpool.tile([P, D], F32)
            nc.scalar.dma_start(we_rep_f[:], gat_we[0:1, :].to_broadcast([P, D]))
            we_rep = to_bf(we_rep_f, [P, D], "cwer")
            we3 = we_rep[:].rearrange("p (o q) -> p o q", o=1)
            att_rep_f = cpool.tile([P, D], F32)
            nc.scalar.dma_start(att_rep_f[:], gat_att[0:1, :].to_broadcast([P, D]))
            att_rep = to_bf(att_rep_f, [P, D], "catr")
            att3 = att_rep[:].rearrange("p (o q) -> p o q", o=1)
            gatb_col = to_bf(load_w(gat_b[:, :], [D, 1], "cgbcf"), [D, 1], "cgbc")
            batch_t = cpool.tile([P, cf.W], BF)
            nc.scalar.dma_start(batch_t[:], batchloc_in[:, :])

            psb = psum.tile([1, HID], F32, space="PSUM", tag="opsum")
            nc.tensor.matmul(out=psb[:, :], lhsT=gatb_col[:], rhs=w1t[1][:], start=True, stop=True)
            b1p = cpool.tile([1, HID], F32)
            nc.vector.tensor_tensor(out=b1p[:], in0=psb[:, :], in1=b1r[:], op=AOT.add)
            b1pb = to_bf(b1p, [1, HID], "cb1pb")

            def make_dinv(deg_dram, pfx):
                degt = cpool.tile([P, cf.W], F32, name=f"{pfx}degt")
                nc.scalar.dma_start(degt[:], deg_dram[:, :])
                m = cpool.tile([P, cf.W], F32, name=f"{pfx}m")
                nc.vector.tensor_scalar_max(out=m[:], in0=degt[:], scalar1=1.0)
                nc.scalar.activation(m[:], m[:], AFT.Sqrt)
                r = cpool.tile([P, cf.W], F32, name=f"{pfx}r")
                nc.vector.reciprocal(out=r[:], in_=m[:])
                nc.vector.tensor_scalar(out=m[:], in0=degt[:], scalar1=0.0,
                                        scalar2=None, op0=AOT.is_gt)
                nc.vector.tensor_tensor(out=r[:], in0=r[:], in1=m[:], op=AOT.mult)
                return r

            dinv_i = make_dinv(deg_in_f, "di_")
            dinv_o = make_dinv(deg_out_f, "do_")
            inv_cnt = cpool.tile([P, cf.GW], F32)
            nc.scalar.dma_start(inv_cnt[:], cnt_f[:, :])
            nc.vector.tensor_scalar_max(out=inv_cnt[:], in0=inv_cnt[:], scalar1=1.0)
            nc.vector.reciprocal(out=inv_cnt[:], in_=inv_cnt[:])

            # ---- phase 1: scaled x tables (t1i/t1o), gl, gr
            for dinv, dst, tag in ((dinv_i, loc["t1i"], "aggA"), (dinv_o, loc["t1o"], "aggB")):
                sc = aggpool.tile([P, cf.W, D], BF, tag=tag)
                nc.scalar.dma_start(sc[:, :, :], x_local[:, :].rearrange("(w p) d -> p w d", p=P))
                nc.vector.tensor_tensor(out=sc[:, :, :], in0=sc[:, :, :],
                                        in1=dinv[:].to_broadcast([P, cf.W, D]), op=AOT.mult)
                nc.sync.dma_start(dst[:, :].rearrange("(w p) d -> p w d", p=P), sc[:, :, :])
            for r in range(cf.W):
                xt = pool.tile([P, D], BF, tag="rowa")
                nc.scalar.dma_start(xt[:], x_local[r * P:(r + 1) * P, :])
                xT = pool.tile([P, P], BF, tag="rowb")
                pe_transpose(xt[:], xT[:])
                for Wmat, dst, tg in ((Wl, loc["gl"], "rowc"), (Wr, gr_loc, "rowd")):
                    ps = psum.tile([P, D], F32, space="PSUM", tag="opsum")
                    nc.tensor.matmul(out=ps[:, :], lhsT=xT[:], rhs=Wmat[:], start=True, stop=True)
                    ot = pool.tile([P, D], BF, tag=tg)
                    nc.vector.tensor_copy(out=ot[:], in_=ps[:, :])
                    nc.scalar.dma_start(dst[r * P:(r + 1) * P, :], ot[:])
            for n in ("t1i", "t1o", "gl"):
                nc.gpsimd.collective_compute("AllGather", AOT.bypass, replica_groups=RG,
                                             ins=[loc[n][:, :]], outs=[tabs[n][:, :]])


            # ---- edge pass emitter
            def edge_pass(ep, pi, table, agg_tag, gat=False):
                ecols = 132 if gat else D
                agg = aggpool.tile([P, cf.W, ecols], F32, tag=agg_tag)
                nc.vector.memset(agg[:, :, :], 0.0)
                dstrel = pool2.tile([P, max(ep.L // P, 1)], BF, tag="dr", bufs=1)
                nc.scalar.dma_start(dstrel[:, :], pi["dstrel"][:, :])
                if gat:
                    eat = pool2.tile([P, max(ep.L // P, 1)], BF, tag="ea", bufs=1)
                    nc.scalar.dma_start(eat[:, :], pi["ea"][:, :])
                call_meta = []
                coff = 0
                for k in range(cf.NCH):
                    for n in ep.calls[k]:
                        call_meta.append((k, n, coff))
                        coff += n
                gtiles = {}

                def get_call(cid):
                    if cid in gtiles:
                        return gtiles[cid]
                    k, n, off = call_meta[cid]
                    nco, sc0 = n // P, off // P
                    it = pool.tile([P, cf.CALL // 16], I16, tag="idxt")
                    nc.scalar.dma_start(it[:, :n // 16], pi["idx"][:, off // 16:(off + n) // 16])
                    g = pool.tile([P, cf.CALL // P, D], BF, tag="gt")
                    tab_view = table[k * cf.CHUNK:min((k + 1) * cf.CHUNK, cf.TAB), :]
                    nc.gpsimd.dma_gather(g[:, :nco, :], tab_view, it[:, :n // 16],
                                         num_idxs=n, num_idxs_reg=n, elem_size=D,
                                         queue_num=cid % 2)
                    oh = pool.tile([P, cf.CALL // P, P], BF, tag="oh")
                    nc.vector.tensor_tensor(
                        out=oh[:, :nco, :],
                        in0=dstrel[:, sc0:sc0 + nco].rearrange("p (s o) -> p s o", o=1)
                            .to_broadcast([P, nco, P]),
                        in1=iota3.to_broadcast([P, nco, P]),
                        op=AOT.is_equal)
                    g2 = None
                    if gat:
                        it2 = pool.tile([P, cf.CALL // 16], I16, tag="idxt2")
                        nc.scalar.dma_start(it2[:, :n // 16], pi["gridx"][:, off // 16:(off + n) // 16])
                        g2 = pool.tile([P, cf.CALL // P, D], BF, tag="gt")
                        nc.gpsimd.dma_gather(g2[:, :nco, :], gr_loc[:, :], it2[:, :n // 16],
                                             num_idxs=n, num_idxs_reg=n, elem_size=D,
                                             queue_num=(cid + 1) % 2)
                    gtiles[cid] = (g, g2, oh)
                    for old in [c for c in gtiles if c < cid - 1]:
                        del gtiles[old]
                    return gtiles[cid]

                for k, w, runs in ep.groups:
                    S = sum(r[1] for r in runs)
                    nmm = S
                    ps = psum.tile([P, ecols], F32, space="PSUM", tag="epsum")
                    mm = 0
                    for (sub0, nsub, cid, col0) in runs:
                        g, g2, oh = get_call(cid)
                        if gat:
                            rhs = pool.tile([P, RUNMAX, 132], BF, tag="gatrhs")
                            gl_s = g[:, col0:col0 + nsub, :]
                            tmp = pool2.tile([P, RUNMAX, D], BF, tag="gtmp")
                            u = pool2.tile([P, RUNMAX, D], BF, tag="gtmp2")
                            tmps = tmp[:, :nsub, :]
                            us = u[:, :nsub, :]
                            nc.vector.tensor_tensor(out=tmps, in0=gl_s,
                                                    in1=g2[:, col0:col0 + nsub, :], op=AOT.add)
                            nc.vector.tensor_tensor(
                                out=us,
                                in0=eat[:, sub0:sub0 + nsub].rearrange("p (s o) -> p s o", o=1).to_broadcast([P, nsub, D]),
                                in1=we3.to_broadcast([P, nsub, D]), op=AOT.mult)
                            nc.vector.tensor_tensor(out=tmps, in0=tmps, in1=us, op=AOT.add)
                            nc.vector.tensor_scalar_mul(out=us, in0=tmps, scalar1=NEG)
                            nc.vector.tensor_tensor(out=tmps, in0=tmps, in1=us, op=AOT.max)
                            nc.vector.tensor_tensor(out=us, in0=tmps,
                                                    in1=att3.to_broadcast([P, nsub, D]), op=AOT.mult)
                            lg = pool.tile([P, RUNMAX, H], F32, tag="lg")
                            nc.vector.tensor_reduce(
                                out=lg[:, :nsub, :],
                                in_=u[:, :nsub, :].rearrange("p s (h c) -> p s h c", h=H),
                                axis=AXX.X, op=AOT.add)
                            nc.scalar.activation(rhs[:, :nsub, D:132], lg[:, :nsub, :], AFT.Exp)
                            nc.vector.tensor_tensor(
                                out=rhs[:, :nsub, 0:D].rearrange("p s (h c) -> p s h c", c=C),
                                in0=gl_s.rearrange("p s (h c) -> p s h c", c=C),
                                in1=rhs[:, :nsub, D:132].rearrange("p s (h o) -> p s h o", o=1).to_broadcast([P, nsub, H, C]),
                                op=AOT.mult)
                            for j in range(nsub):
                                nc.tensor.matmul(out=ps[:, :], lhsT=oh[:, col0 + j, :],
                                                 rhs=rhs[:, j, :], start=(mm == 0), stop=(mm == nmm - 1),
                                                 skip_group_check=True)
                                mm += 1
                        else:
                            for j in range(nsub):
                                nc.tensor.matmul(out=ps[:, :], lhsT=oh[:, col0 + j, :],
                                                 rhs=g[:, col0 + j, :], start=(mm == 0), stop=(mm == nmm - 1),
                                                 skip_group_check=True)
                                mm += 1
                    nc.vector.tensor_tensor(out=agg[:, w, :], in0=agg[:, w, :], in1=ps[:, :], op=AOT.add)
                return agg

            # ---- GCN layers
            def gcn_layer(tin, tout, layer, per_row):
                agg_i = edge_pass(passes["in"], pin["in"], tin, "aggA")
                agg_o = edge_pass(passes["out"], pin["out"], tout, "aggB")
                li = layer - 1
                for r in range(cf.W):
                    si = pool.tile([P, D], BF, tag="rowa")
                    nc.vector.tensor_tensor(out=si[:], in0=agg_i[:, r, 0:D],
                                            in1=dinv_i[:, r:r + 1].to_broadcast([P, D]), op=AOT.mult)
                    so = pool.tile([P, D], BF, tag="rowb")
                    nc.vector.tensor_tensor(out=so[:], in0=agg_o[:, r, 0:D],
                                            in1=dinv_o[:, r:r + 1].to_broadcast([P, D]), op=AOT.mult)
                    siT = pool.tile([P, P], BF, tag="rowc")
                    pe_transpose(si[:], siT[:])
                    soT = pool.tile([P, P], BF, tag="rowd")
                    pe_transpose(so[:], soT[:])
                    ps = psum.tile([P, D], F32, space="PSUM", tag="opsum")
                    nc.tensor.matmul(out=ps[:, :], lhsT=siT[:], rhs=Wi[li][:], start=True, stop=False)
                    nc.tensor.matmul(out=ps[:, :], lhsT=soT[:], rhs=Wo[li][:], start=False, stop=False)
                    nc.tensor.matmul(out=ps[:, :], lhsT=ones_row[:], rhs=bcomb[li][:], start=False, stop=True)
                    ht = pool.tile([P, D], BF, tag="rowe")
                    nc.scalar.activation(ht[:], ps[:, :], AFT.Relu)
                    per_row(r, ht)

            def pool_onehot(r, gw):
                bl = pool.tile([P, 1], BF, tag="bl")
                nc.vector.tensor_scalar_add(out=bl[:], in0=batch_t[:, r:r + 1], scalar1=float(-P * gw))
                po = pool.tile([P, P], BF, tag="po")
                nc.vector.tensor_tensor(out=po[:], in0=bl[:].to_broadcast([P, P]),
                                        in1=iota[:], op=AOT.is_equal)
                return po

            # layer 1 -> t2 tables
            def l1_row(r, ht):
                for dinv, dst, tg in ((dinv_i, loc["t2i"], "rowf"), (dinv_o, loc["t2o"], "rowg")):
                    sc = pool.tile([P, D], BF, tag=tg)
                    nc.vector.tensor_tensor(out=sc[:], in0=ht[:],
                                            in1=dinv[:, r:r + 1].to_broadcast([P, D]), op=AOT.mult)
                    nc.scalar.dma_start(dst[r * P:(r + 1) * P, :], sc[:])

            gcn_layer(tabs["t1i"], tabs["t1o"], 1, l1_row)
            for n in ("t2i", "t2o"):
                nc.gpsimd.collective_compute("AllGather", AOT.bypass, replica_groups=RG,
                                             ins=[loc[n][:, :]], outs=[tabs[n][:, :]])

            # layer 2 -> pool x1
            pool_ps1 = [psum.tile([P, D], F32, space="PSUM", tag="poolps", name=f"poolps1_{gw}") for gw in range(cf.GW)]

            def l2_row(r, ht):
                for gw in range(cf.GW):
                    po = pool_onehot(r, gw)
                    nc.tensor.matmul(out=pool_ps1[gw][:, :], lhsT=po[:], rhs=ht[:],
                                     start=(r == 0), stop=(r == cf.W - 1), skip_group_check=True)

            gcn_layer(tabs["t2i"], tabs["t2o"], 2, l2_row)
            x1part = cpool.tile([P, cf.GW, D], F32)
            for gw in range(cf.GW):
                nc.vector.tensor_copy(out=x1part[:, gw, :], in_=pool_ps1[gw][:, :])

            numden = edge_pass(passes["gat"], pin["gat"], tabs["gl"], "aggA", gat=True)
            pool_ps2 = [psum.tile([P, D], F32, space="PSUM", tag="poolps", name=f"poolps2_{gw}") for gw in range(cf.GW)]
            for r in range(cf.W):
                den = pool.tile([P, H], F32, tag="den")
                nc.vector.tensor_scalar_max(out=den[:], in0=numden[:, r, D:132], scalar1=1e-30)
                nc.vector.reciprocal(out=den[:], in_=den[:])
                o2 = pool.tile([P, D], BF, tag="rowa")
                nc.vector.tensor_tensor(
                    out=o2[:].rearrange("p (h c) -> p h c", c=C),
                    in0=numden[:, r, 0:D].rearrange("p (h c) -> p h c", c=C),
                    in1=den[:].rearrange("p (h o) -> p h o", o=1).to_broadcast([P, H, C]),
                    op=AOT.mult)
                for gw in range(cf.GW):
                    po = pool_onehot(r, gw)
                    nc.tensor.matmul(out=pool_ps2[gw][:, :], lhsT=po[:], rhs=o2[:],
                                     start=(r == 0), stop=(r == cf.W - 1), skip_group_check=True)
            x2part = cpool.tile([P, cf.GW, D], F32)
            for gw in range(cf.GW):
                nc.vector.tensor_copy(out=x2part[:, gw, :], in_=pool_ps2[gw][:, :])

            # ---- pooled AllReduce + head
            nc.sync.dma_start(pool_part[:, 0:D].rearrange("(g p) d -> p g d", p=P), x1part[:, :, :])
            nc.sync.dma_start(pool_part[:, D:2 * D].rearrange("(g p) d -> p g d", p=P), x2part[:, :, :])
            nc.gpsimd.collective_compute("AllReduce", AOT.add, replica_groups=RG,
                                         ins=[pool_part[:, :]], outs=[pool_red[:, :]])
            for gw in range(cf.GW):
                z = pool.tile([P, 2 * D], F32, tag="z")
                nc.scalar.dma_start(z[:], pool_red[gw * P:(gw + 1) * P, :])
                nc.vector.tensor_tensor(
                    out=z[:].rearrange("p (a d) -> p a d", a=2),
                    in0=z[:].rearrange("p (a d) -> p a d", a=2),
                    in1=inv_cnt[:, gw:gw + 1].rearrange("p (o q) -> p o q", q=1).to_broadcast([P, 2, D]),
                    op=AOT.mult)
                zb = pool.tile([P, 2 * D], BF, tag="zb")
                nc.vector.tensor_copy(out=zb[:], in_=z[:])
                ps = psum.tile([P, HID], F32, space="PSUM", tag="opsum")
                for i in range(2):
                    zT = pool.tile([P, P], BF, tag="rowb")
                    pe_transpose(zb[:, i * D:(i + 1) * D], zT[:])
                    nc.tensor.matmul(out=ps[:, :], lhsT=zT[:], rhs=w1t[i][:], start=(i == 0), stop=False)
                nc.tensor.matmul(out=ps[:, :], lhsT=ones_row[:], rhs=b1pb[:], start=False, stop=True)
                zh = pool.tile([P, HID], BF, tag="rowc")
                nc.scalar.activation(zh[:], ps[:, :], AFT.Relu)
                zhT = pool.tile([P, P], BF, tag="rowd")
                pe_transpose(zh[:], zhT[:])
                ps2 = psum.tile([P, CLS], F32, space="PSUM", tag="opsum")
                nc.tensor.matmul(out=ps2[:, :], lhsT=zhT[:], rhs=w2t[:], start=True, stop=False)
                nc.tensor.matmul(out=ps2[:, :], lhsT=ones_row[:], rhs=b2r[:], start=False, stop=True)
                og = pool.tile([P, CLS], F32, tag="roweo")
                nc.vector.tensor_copy(out=og[:], in_=ps2[:, :])
                nc.scalar.dma_start(out_t[gw * P:(gw + 1) * P, :], og[:])

    nc.compile()
    return nc


# ---------------------------------------------------------------- runner

import time
import jax
from jax.sharding import Mesh, PartitionSpec
from jax.experimental.shard_map import shard_map
from concourse.bass2jax import _bass_exec_p, install_neuronx_cc_hook, partition_id_tensor


class BassRunner:
    def __init__(self, nc, n_cores=8):
        install_neuronx_cc_hook()
        self.nc = nc
        self.n_cores = n_cores
        self.partition_name = nc.partition_id_tensor.name if nc.partition_id_tensor else None
        self.dbg_name = nc.dbg_addr.name if nc.dbg_addr is not None else None
        in_names, out_names, out_avals, zero_outs = [], [], [], []
        for alloc in nc.m.functions[0].allocations:
            if not isinstance(alloc, mybir.MemoryLocationSet):
                continue
            name = alloc.memorylocations[0].name
            if alloc.kind == "ExternalInput":
                if name != self.partition_name:
                    in_names.append(name)
            elif alloc.kind == "ExternalOutput":
                shape = tuple(alloc.tensor_shape)
                dtype = mybir.dt.np(alloc.dtype)
                out_names.append(name)
                out_avals.append(jax.core.ShapedArray(shape, dtype))
                zero_outs.append(np.zeros(shape, dtype))
        self.in_names, self.out_names = in_names, out_names
        self.out_avals, self.zero_outs = out_avals, zero_outs
        n_params, n_outs = len(in_names), len(out_avals)
        all_in_names = list(in_names) + list(out_names)
        if self.partition_name is not None:
            all_in_names.append(self.partition_name)
        donate = tuple(range(n_params, n_params + n_outs))

        def _body(*args):
            operands = list(args)
            if self.partition_name is not None:
                operands.append(partition_id_tensor())
            outs = _bass_exec_p.bind(
                *operands, out_avals=tuple(out_avals), in_names=tuple(all_in_names),
                out_names=tuple(out_names), lowering_input_output_aliases=(),
                sim_require_finite=True, sim_require_nnan=True, nc=nc)
            return tuple(outs)

        devices = jax.devices()[:n_cores]
        mesh = Mesh(np.asarray(devices), ("core",))
        in_specs = (PartitionSpec("core"),) * (n_params + n_outs)
        out_specs = (PartitionSpec("core"),) * n_outs
        self.fn = jax.jit(
            shard_map(_body, mesh=mesh, in_specs=in_specs, out_specs=out_specs, check_rep=False),
            donate_argnums=donate, keep_unused=True)

    def run(self, in_maps, repeats=1):
        if self.dbg_name is not None:
            in_maps = [{**m, self.dbg_name: np.zeros((1, 2), np.uint32)} for m in in_maps]
        per_core = [[np.asarray(m[name]) for name in self.in_names] for m in in_maps]
        concat_in = [np.concatenate([per_core[c][i] for c in range(self.n_cores)], axis=0)
                     for i in range(len(self.in_names))]
        import jax as _jax
        from jax.sharding import NamedSharding as _NS, PartitionSpec as _PS
        from jax.sharding import Mesh as _Mesh
        _mesh = _Mesh(np.asarray(_jax.devices()[:self.n_cores]), ("core",))
        concat_in = [_jax.device_put(a, _NS(_mesh, _PS("core"))) for a in concat_in]
        concat_in = _jax.block_until_ready(concat_in)
        from jax.sharding import NamedSharding
        mesh = self.fn_mesh if hasattr(self, "fn_mesh") else None
        times, first_out = [], None
        for _ in range(repeats):
            concat_zeros = [np.zeros((self.n_cores * z.shape[0], *z.shape[1:]), z.dtype)
                            for z in self.zero_outs]
            t0 = time.perf_counter()
            out_arrs = jax.block_until_ready(self.fn(*concat_in, *concat_zeros))
            times.append(time.perf_counter() - t0)
            if first_out is None:
                first_out = [
                    {name: np.asarray(out_arrs[i]).reshape(self.n_cores, *self.out_avals[i].shape)[c]
                     for i, name in enumerate(self.out_names)}
                    for c in range(self.n_cores)]
        return first_out, times


# ---------------------------------------------------------------- entry point

_N, _E, _G = 100000, 1000000, 250


def kernel(**inputs):
    cfg = Cfg(_N, _E, _G)
    in_maps, passes = preprocess(inputs, cfg)
    nc = build_program(cfg, passes)
    r = BassRunner(nc, NCORES)
    outs, _ = r.run(in_maps, repeats=1)
    return np.ascontiguousarray(outs[0]["out"][:_G].astype(np.float32))



# revision 47
# speedup vs baseline: 1.4035x; 1.4035x over previous
"""DirGNN(GCN x2, both directions) + GATv2 + mean-pool + MLP head on 8 trn2 cores.

bf16 variant: tables, gathered rows, one-hot matrices and matmul operands in
bf16 (PSUM/aggregation stays f32). Halves gather/collective bytes and runs the
PE at 4x the fp32 matmul rate.

Sharding: nodes in 8 equal contiguous ranges (batch is sorted, so graphs are
contiguous node ranges). Per core:
  - edge aggregation via dma_gather (256B rows) + one-hot is_equal + PE matmul
    scatter into per-128-node-window PSUM, accumulated in SBUF.
  - tables (dinv-scaled h, gl) are produced locally and AllGather'd.
  - pooling partial sums AllReduce'd; head MLP computed redundantly per core.

Host preprocessing is index/structure only (sorting, bucketing, padding,
int->f32/bf16 encodes); all model FLOPs run on device.
"""
import numpy as np
import ml_dtypes
import concourse.bacc as bacc
import concourse.bass as bass
import concourse.mybir as mybir
import concourse.tile as tile

P = 128
NCORES = 8
D = 128          # feature dim == HC
H, C = 4, 32
CLS = 2
HID = 128
ALPHA = 0.5
NEG = 0.2
F32 = mybir.dt.float32
BF = mybir.dt.bfloat16
NPBF = ml_dtypes.bfloat16
I16 = mybir.dt.int16
AOT = mybir.AluOpType
AFT = mybir.ActivationFunctionType
AXX = mybir.AxisListType


class Cfg:
    def __init__(self, N, E, G, chunk=32768, call=1024):
        self.N, self.E, self.G = N, E, G
        assert N % NCORES == 0
        self.N0 = N // NCORES                       # real nodes per core
        self.PADN0 = ((self.N0 + P - 1) // P) * P   # padded nodes per core
        self.W = self.PADN0 // P                    # node windows per core
        self.TAB = NCORES * self.PADN0              # padded global table rows
        self.CHUNK = chunk
        self.NCH = (self.TAB + chunk - 1) // chunk
        assert chunk <= 32768
        self.GPAD = ((G + P - 1) // P) * P
        self.GW = self.GPAD // P
        self.CALL = call                            # gather idxs per call
        assert call % P == 0


# ---------------------------------------------------------------- host side

def _wrap_idx_call(idx):
    """int array [n] (n%16==0) -> int16 [128, n/16] wrapped + replicated."""
    n = idx.shape[0]
    w = idx.reshape(n // 16, 16).T.astype(np.int16)
    return np.tile(w, (8, 1))


def _pad_to(a, n, val):
    return np.concatenate([a, np.full(n - a.shape[0], val, a.dtype)])


class EdgePass:
    """Host-built, statically-shared layout of one edge pass for all cores."""

    def __init__(self, cfg, src_list, dst_list, ea_list=None):
        cf = cfg
        self.gat = ea_list is not None
        padded = lambda n: (n // cf.N0) * cf.PADN0 + (n % cf.N0)
        percore = []
        counts = np.zeros((NCORES, cf.NCH, cf.W), np.int64)
        for c in range(NCORES):
            src, dst = src_list[c], dst_list[c]
            dstloc = dst - c * cf.N0
            assert (dstloc >= 0).all() and (dstloc < cf.N0).all()
            gidx = padded(src)
            k = gidx // cf.CHUNK
            w = dstloc >> 7
            order = np.lexsort((dstloc, w, k))
            ea = ea_list[c][order] if self.gat else None
            percore.append((gidx[order], dstloc[order], k[order], w[order], ea))
            np.add.at(counts[c], (k[order], w[order]), 1)
        S = np.maximum.reduce([np.ceil(counts[c] / P).astype(np.int64) for c in range(NCORES)])
        self.S = S                                   # [NCH, W] subtiles per group
        self.Smax = int(S.max()) if S.size else 0
        self.L = int(S.sum()) * P                    # padded edges per core
        # call layout per chunk
        self.calls = []                              # per k: list of call sizes
        self.sub2call = []                           # per subtile: (call_id, col)
        call_id = 0
        for k in range(cf.NCH):
            Lk = int(S[k].sum()) * P
            rem = Lk
            sizes = []
            while rem > 0:
                n = min(cf.CALL, rem)
                sizes.append(n)
                for col in range(n // P):
                    self.sub2call.append((call_id, col))
                call_id += 1
                rem -= n
            self.calls.append(sizes)
        self.n_calls = call_id
        self.nsub = self.L // P

        self.idx16, self.dstrel, self.ea, self.gridx16 = [], [], [], []
        for c in range(NCORES):
            gidx, dstloc, kk, ww, ea = percore[c]
            gi_parts, dr_parts, ea_parts, gr_parts = [], [], [], []
            for k in range(cf.NCH):
                for w in range(cf.W):
                    n = int(S[k, w]) * P
                    if n == 0:
                        continue
                    m = (kk == k) & (ww == w)
                    gi_parts.append(_pad_to(gidx[m] - k * cf.CHUNK, n, 0))
                    dr_parts.append(_pad_to((dstloc[m] - w * P).astype(np.float32), n, -1.0))
                    if self.gat:
                        ea_parts.append(_pad_to(ea[m].astype(np.float32), n, 0.0))
                        gr_parts.append(_pad_to(dstloc[m], n, 0))
            gi = np.concatenate(gi_parts) if gi_parts else np.zeros(0, np.int64)
            dr = np.concatenate(dr_parts) if dr_parts else np.zeros(0, np.float32)
            assert gi.shape[0] == self.L
            assert (gi >= 0).all() and (gi < 32768).all()

            def wrap_all(arr):
                off, wraps = 0, []
                for k in range(cf.NCH):
                    for n in self.calls[k]:
                        wraps.append(_wrap_idx_call(arr[off:off + n]))
                        off += n
                return (np.concatenate(wraps, axis=1) if wraps
                        else np.zeros((P, 0), np.int16))

            self.idx16.append(wrap_all(gi))
            self.dstrel.append(np.ascontiguousarray(dr.reshape(-1, P).T).astype(NPBF))
            if self.gat:
                self.ea.append(np.ascontiguousarray(
                    np.concatenate(ea_parts).reshape(-1, P).T).astype(NPBF))
                self.gridx16.append(wrap_all(np.concatenate(gr_parts)))

        # groups: per (k,w) with S>0: list of runs (sub0, nsub, call_id, col0)
        self.groups = []
        sub = 0
        for k in range(cf.NCH):
            for w in range(cf.W):
                s = int(S[k, w])
                if s == 0:
                    continue
                runs, i = [], sub
                while i < sub + s:
                    cid, col = self.sub2call[i]
                    n = 1
                    while i + n < sub + s and self.sub2call[i + n][0] == cid:
                        n += 1
                    runs.append((i, n, cid, col))
                    i += n
                self.groups.append((k, w, runs))
                sub += s
        assert sub == self.nsub


def preprocess(inputs, cfg):
    cf = cfg
    x = np.asarray(inputs["x"], np.float32)
    ei = np.asarray(inputs["edge_index"])
    ea = np.asarray(inputs["edge_attr"], np.float32).reshape(-1)
    batch = np.asarray(inputs["batch"]).astype(np.int64)
    src, dst = ei[0].astype(np.int64), ei[1].astype(np.int64)
    N, G = cf.N, cf.G

    deg_in = np.bincount(dst, minlength=N).astype(np.float32)
    deg_out = np.bincount(src, minlength=N).astype(np.float32)
    cnt = np.bincount(batch, minlength=G).astype(np.float32)

    core_of = lambda n: n // cf.N0
    m_in = [core_of(dst) == c for c in range(NCORES)]
    m_out = [core_of(src) == c for c in range(NCORES)]
    passes = {
        "in": EdgePass(cf, [src[m] for m in m_in], [dst[m] for m in m_in]),
        "out": EdgePass(cf, [dst[m] for m in m_out], [src[m] for m in m_out]),
    }
    gsrc, gdst, gea = [], [], []
    for c in range(NCORES):
        loop = np.arange(c * cf.N0, (c + 1) * cf.N0, dtype=np.int64)
        gsrc.append(np.concatenate([src[m_in[c]], loop]))
        gdst.append(np.concatenate([dst[m_in[c]], loop]))
        gea.append(np.concatenate([ea[m_in[c]], np.ones(cf.N0, np.float32)]))
    passes["gat"] = EdgePass(cf, gsrc, gdst, gea)

    iota = np.tile(np.arange(P, dtype=np.float32)[None, :], (P, 1))
    in_maps = []
    for c in range(NCORES):
        lo, hi = c * cf.N0, (c + 1) * cf.N0
        xl = np.zeros((cf.PADN0, D), NPBF)
        xl[:cf.N0] = x[lo:hi].astype(NPBF)
        nodecol = lambda v, pad: np.ascontiguousarray(
            _pad_to(v[lo:hi].astype(np.float32), cf.PADN0, pad).reshape(cf.W, P).T)
        m = {
            "x_local": xl,
            "deg_in_f": nodecol(deg_in, 0.0),
            "deg_out_f": nodecol(deg_out, 0.0),
            "batchloc": nodecol(batch.astype(np.float32), -1.0).astype(NPBF),
            "cnt_f": np.ascontiguousarray(_pad_to(cnt, cf.GPAD, 0.0).reshape(cf.GW, P).T),
            "iota_in": iota.astype(NPBF),
            "dir_w_in": np.asarray(inputs["dir_w_in"], np.float32).reshape(2 * D, D),
            "dir_b_in": np.asarray(inputs["dir_b_in"], np.float32),
            "dir_w_out": np.asarray(inputs["dir_w_out"], np.float32).reshape(2 * D, D),
            "dir_b_out": np.asarray(inputs["dir_b_out"], np.float32),
            "gat_wl": np.asarray(inputs["gat_wl"], np.float32),
            "gat_wr": np.asarray(inputs["gat_wr"], np.float32),
            "gat_we": np.asarray(inputs["gat_we"], np.float32),
            "gat_att": np.asarray(inputs["gat_att"], np.float32).reshape(1, D),
            "gat_b": np.asarray(inputs["gat_b"], np.float32).reshape(D, 1),
            "w1": np.asarray(inputs["w1"], np.float32),
            "b1": np.asarray(inputs["b1"], np.float32).reshape(1, HID),
            "w2": np.asarray(inputs["w2"], np.float32),
            "b2": np.asarray(inputs["b2"], np.float32).reshape(1, CLS),
        }
        for pname, ep in passes.items():
            m[f"idx_{pname}"] = ep.idx16[c]
            m[f"dstrel_{pname}"] = ep.dstrel[c]
            if ep.gat:
                m[f"ea_{pname}"] = ep.ea[c]
                m[f"gridx_{pname}"] = ep.gridx16[c]
        in_maps.append(m)
    return in_maps, passes


# ---------------------------------------------------------------- device side

def build_program(cfg, passes, stop_after=None):
    stop_after = None
    cf = cfg
    nc = bacc.Bacc("TRN2", target_bir_lowering=False, debug=False, num_devices=NCORES,
                   num_swdge_queues=2)

    def inp(name, shape, dt=F32):
        return nc.dram_tensor(name, shape, dt, kind="ExternalInput")

    x_local = inp("x_local", [cf.PADN0, D], BF)
    deg_in_f = inp("deg_in_f", [P, cf.W])
    deg_out_f = inp("deg_out_f", [P, cf.W])
    batchloc_in = inp("batchloc", [P, cf.W], BF)
    cnt_f = inp("cnt_f", [P, cf.GW])
    iota_in = inp("iota_in", [P, P], BF)
    dir_w_in = inp("dir_w_in", [2 * D, D])
    dir_b_in = inp("dir_b_in", [2, D])
    dir_w_out = inp("dir_w_out", [2 * D, D])
    dir_b_out = inp("dir_b_out", [2, D])
    gat_wl = inp("gat_wl", [D, D])
    gat_wr = inp("gat_wr", [D, D])
    gat_we = inp("gat_we", [1, D])
    gat_att = inp("gat_att", [1, D])
    gat_b = inp("gat_b", [D, 1])
    w1_in = inp("w1", [2 * D, HID])
    b1_in = inp("b1", [1, HID])
    w2_in = inp("w2", [HID, CLS])
    b2_in = inp("b2", [1, CLS])
    pin = {}
    for pname, ep in passes.items():
        pin[pname] = {
            "idx": inp(f"idx_{pname}", [P, ep.L // 16], I16),
            "dstrel": inp(f"dstrel_{pname}", [P, ep.L // P], BF),
        }
        if ep.gat:
            pin[pname]["ea"] = inp(f"ea_{pname}", [P, ep.L // P], BF)
            pin[pname]["gridx"] = inp(f"gridx_{pname}", [P, ep.L // 16], I16)

    out_t = nc.dram_tensor("out", [cf.GPAD, CLS], F32, kind="ExternalOutput")

    loc = {n: nc.dram_tensor(f"{n}_loc", [cf.PADN0, D], BF, kind="Internal")
           for n in ("t1i", "t1o", "t2i", "t2o", "gl")}
    gr_loc = nc.dram_tensor("gr_loc", [cf.PADN0, D], BF, kind="Internal")
    tabs = {n: nc.dram_tensor(f"{n}_tab", [cf.TAB, D], BF, kind="Internal", addr_space="Shared")
            for n in ("t1i", "t1o", "t2i", "t2o", "gl")}
    pool_part = nc.dram_tensor("pool_part", [cf.GPAD, 2 * D], F32, kind="Internal")
    pool_red = nc.dram_tensor("pool_red", [cf.GPAD, 2 * D], F32, kind="Internal", addr_space="Shared")
    RG = [list(range(NCORES))]
    Smax = max(ep.Smax for ep in passes.values())
    RUNMAX = Smax

    with tile.TileContext(nc) as tc:
        with (
            tc.tile_pool(name="const", bufs=1) as cpool,
            tc.tile_pool(name="agg", bufs=1) as aggpool,
            tc.tile_pool(name="work", bufs=3) as pool,
            tc.tile_pool(name="work2", bufs=2) as pool2,
            tc.tile_pool(name="psum", bufs=2, space="PSUM") as psum,
        ):
            # ---- constants / weights
            iota = cpool.tile([P, P], BF)
            nc.scalar.dma_start(iota[:], iota_in[:, :])
            iota3 = iota[:].rearrange("p (o q) -> p o q", o=1)
            ones_row = cpool.tile([1, P], BF)
            nc.vector.memset(ones_row[:], 1.0)
            from concourse.masks import make_identity
            ident = cpool.tile([P, P], BF)
            make_identity(nc, ident[:])

            def pe_transpose(src_ap, out_tile_ap):
                tp = psum.tile([P, P], BF, space="PSUM", tag="tpsum", name="tp")
                nc.tensor.transpose(out=tp[:, :], in_=src_ap, identity=ident[:])
                nc.vector.tensor_copy(out=out_tile_ap, in_=tp[:, :])

            def load_w(src_ap, shape, name, dt=F32):
                t = cpool.tile(shape, dt, name=name)
                nc.scalar.dma_start(t[:], src_ap)
                return t

            def to_bf(t, shape, name):
                b = cpool.tile(shape, BF, name=name)
                nc.vector.tensor_copy(out=b[:], in_=t[:])
                return b

            Wi_f = [load_w(dir_w_in[i * D:(i + 1) * D, :], [D, D], f"cwif{i}") for i in range(2)]
            Wo_f = [load_w(dir_w_out[i * D:(i + 1) * D, :], [D, D], f"cwof{i}") for i in range(2)]
            for i in range(2):
                nc.vector.tensor_scalar_mul(out=Wi_f[i][:], in0=Wi_f[i][:], scalar1=1.0 - ALPHA)
                nc.vector.tensor_scalar_mul(out=Wo_f[i][:], in0=Wo_f[i][:], scalar1=ALPHA)
            Wi = [to_bf(Wi_f[i], [D, D], f"cwi{i}") for i in range(2)]
            Wo = [to_bf(Wo_f[i], [D, D], f"cwo{i}") for i in range(2)]
            bcomb = []
            for i in range(2):
                bi = load_w(dir_b_in[i:i + 1, :], [1, D], f"cbif{i}")
                bo = load_w(dir_b_out[i:i + 1, :], [1, D], f"cbof{i}")
                nc.vector.tensor_scalar_mul(out=bi[:], in0=bi[:], scalar1=1.0 - ALPHA)
                nc.vector.tensor_scalar_mul(out=bo[:], in0=bo[:], scalar1=ALPHA)
                nc.vector.tensor_tensor(out=bi[:], in0=bi[:], in1=bo[:], op=AOT.add)
                bcomb.append(to_bf(bi, [1, D], f"cbi{i}"))
            Wl = to_bf(load_w(gat_wl[:, :], [D, D], "cwlf"), [D, D], "cwl")
            Wr = to_bf(load_w(gat_wr[:, :], [D, D], "cwrf"), [D, D], "cwr")
            w1t = [to_bf(load_w(w1_in[i * D:(i + 1) * D, :], [D, HID], f"cw1f{i}"), [D, HID], f"cw1{i}")
                   for i in range(2)]
            b1r = load_w(b1_in[:, :], [1, HID], "cb1r")
            w2t = to_bf(load_w(w2_in[:, :], [HID, CLS], "cw2f"), [HID, CLS], "cw2t")
            b2r = to_bf(load_w(b2_in[:, :], [1, CLS], "cb2f"), [1, CLS], "cb2r")
            we_rep_f = cpool.tile([P, D], F32)
            nc.scalar.dma_start(we_rep_f[:], gat_we[0:1, :].to_broadcast([P, D]))
            we_rep = to_bf(we_rep_f, [P, D], "cwer")
            we3 = we_rep[:].rearrange("p (o q) -> p o q", o=1)
            att_rep_f = cpool.tile([P, D], F32)
            nc.scalar.dma_start(att_rep_f[:], gat_att[0:1, :].to_broadcast([P, D]))
            att_rep = to_bf(att_rep_f, [P, D], "catr")
            att3 = att_rep[:].rearrange("p (o q) -> p o q", o=1)
            gatb_col = to_bf(load_w(gat_b[:, :], [D, 1], "cgbcf"), [D, 1], "cgbc")
            batch_t = cpool.tile([P, cf.W], BF)
            nc.scalar.dma_start(batch_t[:], batchloc_in[:, :])

            psb = psum.tile([1, HID], F32, space="PSUM", tag="opsum")
            nc.tensor.matmul(out=psb[:, :], lhsT=gatb_col[:], rhs=w1t[1][:], start=True, stop=True)
            b1p = cpool.tile([1, HID], F32)
            nc.vector.tensor_tensor(out=b1p[:], in0=psb[:, :], in1=b1r[:], op=AOT.add)
            b1pb = to_bf(b1p, [1, HID], "cb1pb")

            def make_dinv(deg_dram, pfx):
                degt = cpool.tile([P, cf.W], F32, name=f"{pfx}degt")
                nc.scalar.dma_start(degt[:], deg_dram[:, :])
                m = cpool.tile([P, cf.W], F32, name=f"{pfx}m")
                nc.vector.tensor_scalar_max(out=m[:], in0=degt[:], scalar1=1.0)
                nc.scalar.activation(m[:], m[:], AFT.Sqrt)
                r = cpool.tile([P, cf.W], F32, name=f"{pfx}r")
                nc.vector.reciprocal(out=r[:], in_=m[:])
                nc.vector.tensor_scalar(out=m[:], in0=degt[:], scalar1=0.0,
                                        scalar2=None, op0=AOT.is_gt)
                nc.vector.tensor_tensor(out=r[:], in0=r[:], in1=m[:], op=AOT.mult)
                return r

            dinv_i = make_dinv(deg_in_f, "di_")
            dinv_o = make_dinv(deg_out_f, "do_")
            inv_cnt = cpool.tile([P, cf.GW], F32)
            nc.scalar.dma_start(inv_cnt[:], cnt_f[:, :])
            nc.vector.tensor_scalar_max(out=inv_cnt[:], in0=inv_cnt[:], scalar1=1.0)
            nc.vector.reciprocal(out=inv_cnt[:], in_=inv_cnt[:])

            # ---- phase 1: scaled x tables (t1i/t1o), gl, gr
            for dinv, dst, tag in ((dinv_i, loc["t1i"], "aggA"), (dinv_o, loc["t1o"], "aggB")):
                sc = aggpool.tile([P, cf.W, D], BF, tag=tag)
                nc.scalar.dma_start(sc[:, :, :], x_local[:, :].rearrange("(w p) d -> p w d", p=P))
                nc.vector.tensor_tensor(out=sc[:, :, :], in0=sc[:, :, :],
                                        in1=dinv[:].to_broadcast([P, cf.W, D]), op=AOT.mult)
                nc.sync.dma_start(dst[:, :].rearrange("(w p) d -> p w d", p=P), sc[:, :, :])
            for r in range(cf.W):
                xt = pool.tile([P, D], BF, tag="rowa")
                nc.scalar.dma_start(xt[:], x_local[r * P:(r + 1) * P, :])
                xT = pool.tile([P, P], BF, tag="rowb")
                pe_transpose(xt[:], xT[:])
                for Wmat, dst, tg in ((Wl, loc["gl"], "rowc"), (Wr, gr_loc, "rowd")):
                    ps = psum.tile([P, D], F32, space="PSUM", tag="opsum")
                    nc.tensor.matmul(out=ps[:, :], lhsT=xT[:], rhs=Wmat[:], start=True, stop=True)
                    ot = pool.tile([P, D], BF, tag=tg)
                    nc.vector.tensor_copy(out=ot[:], in_=ps[:, :])
                    nc.scalar.dma_start(dst[r * P:(r + 1) * P, :], ot[:])
            for n in ("t1i", "t1o", "gl"):
                nc.gpsimd.collective_compute("AllGather", AOT.bypass, replica_groups=RG,
                                             ins=[loc[n][:, :]], outs=[tabs[n][:, :]])


            # ---- edge pass emitter
            def edge_pass(ep, pi, table, agg_tag, gat=False):
                ecols = 132 if gat else D
                agg = aggpool.tile([P, cf.W, ecols], F32, tag=agg_tag)
                nc.vector.memset(agg[:, :, :], 0.0)
                dstrel = pool2.tile([P, max(ep.L // P, 1)], BF, tag="dr", bufs=1)
                nc.scalar.dma_start(dstrel[:, :], pi["dstrel"][:, :])
                if gat:
                    eat = pool2.tile([P, max(ep.L // P, 1)], BF, tag="ea", bufs=1)
                    nc.scalar.dma_start(eat[:, :], pi["ea"][:, :])
                call_meta = []
                coff = 0
                for k in range(cf.NCH):
                    for n in ep.calls[k]:
                        call_meta.append((k, n, coff))
                        coff += n
                gtiles = {}

                def get_call(cid):
                    if cid in gtiles:
                        return gtiles[cid]
                    k, n, off = call_meta[cid]
                    it = pool.tile([P, cf.CALL // 16], I16, tag="idxt")
                    nc.scalar.dma_start(it[:, :n // 16], pi["idx"][:, off // 16:(off + n) // 16])
                    g = pool.tile([P, cf.CALL // P, D], BF, tag="gt")
                    tab_view = table[k * cf.CHUNK:min((k + 1) * cf.CHUNK, cf.TAB), :]
                    nc.gpsimd.dma_gather(g[:, :n // P, :], tab_view, it[:, :n // 16],
                                         num_idxs=n, num_idxs_reg=n, elem_size=D,
                                         queue_num=cid % 2)
                    g2 = None
                    if gat:
                        it2 = pool.tile([P, cf.CALL // 16], I16, tag="idxt2")
                        nc.scalar.dma_start(it2[:, :n // 16], pi["gridx"][:, off // 16:(off + n) // 16])
                        g2 = pool.tile([P, cf.CALL // P, D], BF, tag="gt")
                        nc.gpsimd.dma_gather(g2[:, :n // P, :], gr_loc[:, :], it2[:, :n // 16],
                                             num_idxs=n, num_idxs_reg=n, elem_size=D,
                                             queue_num=(cid + 1) % 2)
                    gtiles[cid] = (g, g2)
                    for old in [c for c in gtiles if c < cid - 1]:
                        del gtiles[old]
                    return gtiles[cid]

                for k, w, runs in ep.groups:
                    s0 = runs[0][0]
                    S = sum(r[1] for r in runs)
                    nmm = S
                    oh = pool.tile([P, Smax, P], BF, tag="oh")
                    nc.vector.tensor_tensor(
                        out=oh[:, :S, :],
                        in0=dstrel[:, s0:s0 + S].rearrange("p (s o) -> p s o", o=1).to_broadcast([P, S, P]),
                        in1=iota3.to_broadcast([P, S, P]),
                        op=AOT.is_equal)
                    ps = psum.tile([P, ecols], F32, space="PSUM", tag="epsum")
                    mm = 0
                    for (sub0, nsub, cid, col0) in runs:
                        g, g2 = get_call(cid)
                        if gat:
                            rhs = pool.tile([P, RUNMAX, 132], BF, tag="gatrhs")
                            gl_s = g[:, col0:col0 + nsub, :]
                            tmp = pool2.tile([P, RUNMAX, D], BF, tag="gtmp")
                            u = pool2.tile([P, RUNMAX, D], BF, tag="gtmp2")
                            tmps = tmp[:, :nsub, :]
                            us = u[:, :nsub, :]
                            nc.vector.tensor_tensor(out=tmps, in0=gl_s,
                                                    in1=g2[:, col0:col0 + nsub, :], op=AOT.add)
                            nc.vector.tensor_tensor(
                                out=us,
                                in0=eat[:, sub0:sub0 + nsub].rearrange("p (s o) -> p s o", o=1).to_broadcast([P, nsub, D]),
                                in1=we3.to_broadcast([P, nsub, D]), op=AOT.mult)
                            nc.vector.tensor_tensor(out=tmps, in0=tmps, in1=us, op=AOT.add)
                            nc.vector.tensor_scalar_mul(out=us, in0=tmps, scalar1=NEG)
                            nc.vector.tensor_tensor(out=tmps, in0=tmps, in1=us, op=AOT.max)
                            nc.vector.tensor_tensor(out=us, in0=tmps,
                                                    in1=att3.to_broadcast([P, nsub, D]), op=AOT.mult)
                            lg = pool.tile([P, RUNMAX, H], F32, tag="lg")
                            nc.vector.tensor_reduce(
                                out=lg[:, :nsub, :],
                                in_=u[:, :nsub, :].rearrange("p s (h c) -> p s h c", h=H),
                                axis=AXX.X, op=AOT.add)
                            nc.scalar.activation(rhs[:, :nsub, D:132], lg[:, :nsub, :], AFT.Exp)
                            nc.vector.tensor_tensor(
                                out=rhs[:, :nsub, 0:D].rearrange("p s (h c) -> p s h c", c=C),
                                in0=gl_s.rearrange("p s (h c) -> p s h c", c=C),
                                in1=rhs[:, :nsub, D:132].rearrange("p s (h o) -> p s h o", o=1).to_broadcast([P, nsub, H, C]),
                                op=AOT.mult)
                            for j in range(nsub):
                                nc.tensor.matmul(out=ps[:, :], lhsT=oh[:, sub0 - s0 + j, :],
                                                 rhs=rhs[:, j, :], start=(mm == 0), stop=(mm == nmm - 1),
                                                 skip_group_check=True)
                                mm += 1
                        else:
                            for j in range(nsub):
                                nc.tensor.matmul(out=ps[:, :], lhsT=oh[:, sub0 - s0 + j, :],
                                                 rhs=g[:, col0 + j, :], start=(mm == 0), stop=(mm == nmm - 1),
                                                 skip_group_check=True)
                                mm += 1
                    nc.vector.tensor_tensor(out=agg[:, w, :], in0=agg[:, w, :], in1=ps[:, :], op=AOT.add)
                return agg

            # ---- GCN layers
            def gcn_layer(tin, tout, layer, per_row):
                agg_i = edge_pass(passes["in"], pin["in"], tin, "aggA")
                agg_o = edge_pass(passes["out"], pin["out"], tout, "aggB")
                li = layer - 1
                for r in range(cf.W):
                    si = pool.tile([P, D], BF, tag="rowa")
                    nc.vector.tensor_tensor(out=si[:], in0=agg_i[:, r, 0:D],
                                            in1=dinv_i[:, r:r + 1].to_broadcast([P, D]), op=AOT.mult)
                    so = pool.tile([P, D], BF, tag="rowb")
                    nc.vector.tensor_tensor(out=so[:], in0=agg_o[:, r, 0:D],
                                            in1=dinv_o[:, r:r + 1].to_broadcast([P, D]), op=AOT.mult)
                    siT = pool.tile([P, P], BF, tag="rowc")
                    pe_transpose(si[:], siT[:])
                    soT = pool.tile([P, P], BF, tag="rowd")
                    pe_transpose(so[:], soT[:])
                    ps = psum.tile([P, D], F32, space="PSUM", tag="opsum")
                    nc.tensor.matmul(out=ps[:, :], lhsT=siT[:], rhs=Wi[li][:], start=True, stop=False)
                    nc.tensor.matmul(out=ps[:, :], lhsT=soT[:], rhs=Wo[li][:], start=False, stop=False)
                    nc.tensor.matmul(out=ps[:, :], lhsT=ones_row[:], rhs=bcomb[li][:], start=False, stop=True)
                    ht = pool.tile([P, D], BF, tag="rowe")
                    nc.scalar.activation(ht[:], ps[:, :], AFT.Relu)
                    per_row(r, ht)

            def pool_onehot(r, gw):
                bl = pool.tile([P, 1], BF, tag="bl")
                nc.vector.tensor_scalar_add(out=bl[:], in0=batch_t[:, r:r + 1], scalar1=float(-P * gw))
                po = pool.tile([P, P], BF, tag="po")
                nc.vector.tensor_tensor(out=po[:], in0=bl[:].to_broadcast([P, P]),
                                        in1=iota[:], op=AOT.is_equal)
                return po

            # layer 1 -> t2 tables
            def l1_row(r, ht):
                for dinv, dst, tg in ((dinv_i, loc["t2i"], "rowf"), (dinv_o, loc["t2o"], "rowg")):
                    sc = pool.tile([P, D], BF, tag=tg)
                    nc.vector.tensor_tensor(out=sc[:], in0=ht[:],
                                            in1=dinv[:, r:r + 1].to_broadcast([P, D]), op=AOT.mult)
                    nc.scalar.dma_start(dst[r * P:(r + 1) * P, :], sc[:])

            gcn_layer(tabs["t1i"], tabs["t1o"], 1, l1_row)
            for n in ("t2i", "t2o"):
                nc.gpsimd.collective_compute("AllGather", AOT.bypass, replica_groups=RG,
                                             ins=[loc[n][:, :]], outs=[tabs[n][:, :]])

            # layer 2 -> pool x1
            pool_ps1 = [psum.tile([P, D], F32, space="PSUM", tag="poolps", name=f"poolps1_{gw}") for gw in range(cf.GW)]

            def l2_row(r, ht):
                for gw in range(cf.GW):
                    po = pool_onehot(r, gw)
                    nc.tensor.matmul(out=pool_ps1[gw][:, :], lhsT=po[:], rhs=ht[:],
                                     start=(r == 0), stop=(r == cf.W - 1), skip_group_check=True)

            gcn_layer(tabs["t2i"], tabs["t2o"], 2, l2_row)
            x1part = cpool.tile([P, cf.GW, D], F32)
            for gw in range(cf.GW):
                nc.vector.tensor_copy(out=x1part[:, gw, :], in_=pool_ps1[gw][:, :])

            numden = edge_pass(passes["gat"], pin["gat"], tabs["gl"], "aggA", gat=True)
            pool_ps2 = [psum.tile([P, D], F32, space="PSUM", tag="poolps", name=f"poolps2_{gw}") for gw in range(cf.GW)]
            for r in range(cf.W):
                den = pool.tile([P, H], F32, tag="den")
                nc.vector.tensor_scalar_max(out=den[:], in0=numden[:, r, D:132], scalar1=1e-30)
                nc.vector.reciprocal(out=den[:], in_=den[:])
                o2 = pool.tile([P, D], BF, tag="rowa")
                nc.vector.tensor_tensor(
                    out=o2[:].rearrange("p (h c) -> p h c", c=C),
                    in0=numden[:, r, 0:D].rearrange("p (h c) -> p h c", c=C),
                    in1=den[:].rearrange("p (h o) -> p h o", o=1).to_broadcast([P, H, C]),
                    op=AOT.mult)
                for gw in range(cf.GW):
                    po = pool_onehot(r, gw)
                    nc.tensor.matmul(out=pool_ps2[gw][:, :], lhsT=po[:], rhs=o2[:],
                                     start=(r == 0), stop=(r == cf.W - 1), skip_group_check=True)
            x2part = cpool.tile([P, cf.GW, D], F32)
            for gw in range(cf.GW):
                nc.vector.tensor_copy(out=x2part[:, gw, :], in_=pool_ps2[gw][:, :])

            # ---- pooled AllReduce + head
            nc.sync.dma_start(pool_part[:, 0:D].rearrange("(g p) d -> p g d", p=P), x1part[:, :, :])
            nc.sync.dma_start(pool_part[:, D:2 * D].rearrange("(g p) d -> p g d", p=P), x2part[:, :, :])
            nc.gpsimd.collective_compute("AllReduce", AOT.add, replica_groups=RG,
                                         ins=[pool_part[:, :]], outs=[pool_red[:, :]])
            for gw in range(cf.GW):
                z = pool.tile([P, 2 * D], F32, tag="z")
                nc.scalar.dma_start(z[:], pool_red[gw * P:(gw + 1) * P, :])
                nc.vector.tensor_tensor(
                    out=z[:].rearrange("p (a d) -> p a d", a=2),
                    in0=z[:].rearrange("p (a d) -> p a d", a=2),
                    in1=inv_cnt[:, gw:gw + 1].rearrange("p (o q) -> p o q", q=1).to_broadcast([P, 2, D]),
                    op=AOT.mult)
                zb = pool.tile([P, 2 * D], BF, tag="zb")
                nc.vector.tensor_copy(out=zb[:], in_=z[:])
                ps = psum.tile([P, HID], F32, space="PSUM", tag="opsum")
                for i in range(2):
                    zT = pool.tile([P, P], BF, tag="rowb")
                    pe_transpose(zb[:, i * D:(i + 1) * D], zT[:])
                    nc.tensor.matmul(out=ps[:, :], lhsT=zT[:], rhs=w1t[i][:], start=(i == 0), stop=False)
                nc.tensor.matmul(out=ps[:, :], lhsT=ones_row[:], rhs=b1pb[:], start=False, stop=True)
                zh = pool.tile([P, HID], BF, tag="rowc")
                nc.scalar.activation(zh[:], ps[:, :], AFT.Relu)
                zhT = pool.tile([P, P], BF, tag="rowd")
                pe_transpose(zh[:], zhT[:])
                ps2 = psum.tile([P, CLS], F32, space="PSUM", tag="opsum")
                nc.tensor.matmul(out=ps2[:, :], lhsT=zhT[:], rhs=w2t[:], start=True, stop=False)
                nc.tensor.matmul(out=ps2[:, :], lhsT=ones_row[:], rhs=b2r[:], start=False, stop=True)
                og = pool.tile([P, CLS], F32, tag="roweo")
                nc.vector.tensor_copy(out=og[:], in_=ps2[:, :])
                nc.scalar.dma_start(out_t[gw * P:(gw + 1) * P, :], og[:])

    nc.compile()
    return nc


# ---------------------------------------------------------------- runner

import time
import jax
from jax.sharding import Mesh, PartitionSpec
from jax.experimental.shard_map import shard_map
from concourse.bass2jax import _bass_exec_p, install_neuronx_cc_hook, partition_id_tensor


class BassRunner:
    def __init__(self, nc, n_cores=8):
        install_neuronx_cc_hook()
        self.nc = nc
        self.n_cores = n_cores
        self.partition_name = nc.partition_id_tensor.name if nc.partition_id_tensor else None
        self.dbg_name = nc.dbg_addr.name if nc.dbg_addr is not None else None
        in_names, out_names, out_avals, zero_outs = [], [], [], []
        for alloc in nc.m.functions[0].allocations:
            if not isinstance(alloc, mybir.MemoryLocationSet):
                continue
            name = alloc.memorylocations[0].name
            if alloc.kind == "ExternalInput":
                if name != self.partition_name:
                    in_names.append(name)
            elif alloc.kind == "ExternalOutput":
                shape = tuple(alloc.tensor_shape)
                dtype = mybir.dt.np(alloc.dtype)
                out_names.append(name)
                out_avals.append(jax.core.ShapedArray(shape, dtype))
                zero_outs.append(np.zeros(shape, dtype))
        self.in_names, self.out_names = in_names, out_names
        self.out_avals, self.zero_outs = out_avals, zero_outs
        n_params, n_outs = len(in_names), len(out_avals)
        all_in_names = list(in_names) + list(out_names)
        if self.partition_name is not None:
            all_in_names.append(self.partition_name)
        donate = tuple(range(n_params, n_params + n_outs))

        def _body(*args):
            operands = list(args)
            if self.partition_name is not None:
                operands.append(partition_id_tensor())
            outs = _bass_exec_p.bind(
                *operands, out_avals=tuple(out_avals), in_names=tuple(all_in_names),
                out_names=tuple(out_names), lowering_input_output_aliases=(),
                sim_require_finite=True, sim_require_nnan=True, nc=nc)
            return tuple(outs)

        devices = jax.devices()[:n_cores]
        mesh = Mesh(np.asarray(devices), ("core",))
        in_specs = (PartitionSpec("core"),) * (n_params + n_outs)
        out_specs = (PartitionSpec("core"),) * n_outs
        self.fn = jax.jit(
            shard_map(_body, mesh=mesh, in_specs=in_specs, out_specs=out_specs, check_rep=False),
            donate_argnums=donate, keep_unused=True)

    def run(self, in_maps, repeats=1):
        if self.dbg_name is not None:
            in_maps = [{**m, self.dbg_name: np.zeros((1, 2), np.uint32)} for m in in_maps]
        per_core = [[np.asarray(m[name]) for name in self.in_names] for m in in_maps]
        concat_in = [np.concatenate([per_core[c][i] for c in range(self.n_cores)], axis=0)
                     for i in range(len(self.in_names))]
        import jax as _jax
        from jax.sharding import NamedSharding as _NS, PartitionSpec as _PS
        from jax.sharding import Mesh as _Mesh
        _mesh = _Mesh(np.asarray(_jax.devices()[:self.n_cores]), ("core",))
        concat_in = [_jax.device_put(a, _NS(_mesh, _PS("core"))) for a in concat_in]
        concat_in = _jax.block_until_ready(concat_in)
        from jax.sharding import NamedSharding
        mesh = self.fn_mesh if hasattr(self, "fn_mesh") else None
        times, first_out = [], None
        for _ in range(repeats):
            concat_zeros = [np.zeros((self.n_cores * z.shape[0], *z.shape[1:]), z.dtype)
                            for z in self.zero_outs]
            t0 = time.perf_counter()
            out_arrs = jax.block_until_ready(self.fn(*concat_in, *concat_zeros))
            times.append(time.perf_counter() - t0)
            if first_out is None:
                first_out = [
                    {name: np.asarray(out_arrs[i]).reshape(self.n_cores, *self.out_avals[i].shape)[c]
                     for i, name in enumerate(self.out_names)}
                    for c in range(self.n_cores)]
        return first_out, times


# ---------------------------------------------------------------- entry point

_N, _E, _G = 100000, 1000000, 250


def kernel(**inputs):
    cfg = Cfg(_N, _E, _G)
    in_maps, passes = preprocess(inputs, cfg)
    nc = build_program(cfg, passes)
    r = BassRunner(nc, NCORES)
    outs, _ = r.run(in_maps, repeats=1)
    return np.ascontiguousarray(outs[0]["out"][:_G].astype(np.float32))



# revision 48
# speedup vs baseline: 1.7226x; 1.2274x over previous
"""DirGNN(GCN x2, both directions) + GATv2 + mean-pool + MLP head on 8 trn2 cores.

bf16 variant: tables, gathered rows, one-hot matrices and matmul operands in
bf16 (PSUM/aggregation stays f32). Halves gather/collective bytes and runs the
PE at 4x the fp32 matmul rate.

Sharding: nodes in 8 equal contiguous ranges (batch is sorted, so graphs are
contiguous node ranges). Per core:
  - edge aggregation via dma_gather (256B rows) + one-hot is_equal + PE matmul
    scatter into per-128-node-window PSUM, accumulated in SBUF.
  - tables (dinv-scaled h, gl) are produced locally and AllGather'd.
  - pooling partial sums AllReduce'd; head MLP computed redundantly per core.

Host preprocessing is index/structure only (sorting, bucketing, padding,
int->f32/bf16 encodes); all model FLOPs run on device.
"""
import numpy as np
import ml_dtypes
import concourse.bacc as bacc
import concourse.bass as bass
import concourse.mybir as mybir
import concourse.tile as tile

P = 128
NCORES = 8
D = 128          # feature dim == HC
H, C = 4, 32
CLS = 2
HID = 128
ALPHA = 0.5
NEG = 0.2
F32 = mybir.dt.float32
BF = mybir.dt.bfloat16
NPBF = ml_dtypes.bfloat16
I16 = mybir.dt.int16
AOT = mybir.AluOpType
AFT = mybir.ActivationFunctionType
AXX = mybir.AxisListType


class Cfg:
    def __init__(self, N, E, G, chunk=32768, call=1024):
        self.N, self.E, self.G = N, E, G
        assert N % NCORES == 0
        self.N0 = N // NCORES                       # real nodes per core
        self.PADN0 = ((self.N0 + P - 1) // P) * P   # padded nodes per core
        self.W = self.PADN0 // P                    # node windows per core
        self.TAB = NCORES * self.PADN0              # padded global table rows
        self.CHUNK = chunk
        self.NCH = (self.TAB + chunk - 1) // chunk
        assert chunk <= 32768
        self.GPAD = ((G + P - 1) // P) * P
        self.GW = self.GPAD // P
        self.CALL = call                            # gather idxs per call
        assert call % P == 0


# ---------------------------------------------------------------- host side

def _wrap_idx_call(idx):
    """int array [n] (n%16==0) -> int16 [128, n/16] wrapped + replicated."""
    n = idx.shape[0]
    w = idx.reshape(n // 16, 16).T.astype(np.int16)
    return np.tile(w, (8, 1))


def _pad_to(a, n, val):
    return np.concatenate([a, np.full(n - a.shape[0], val, a.dtype)])


class EdgePass:
    """Host-built, statically-shared layout of one edge pass for all cores."""

    def __init__(self, cfg, src_list, dst_list, ea_list=None):
        cf = cfg
        self.gat = ea_list is not None
        padded = lambda n: (n // cf.N0) * cf.PADN0 + (n % cf.N0)
        percore = []
        counts = np.zeros((NCORES, cf.NCH, cf.W), np.int64)
        for c in range(NCORES):
            src, dst = src_list[c], dst_list[c]
            dstloc = dst - c * cf.N0
            assert (dstloc >= 0).all() and (dstloc < cf.N0).all()
            gidx = padded(src)
            k = gidx // cf.CHUNK
            w = dstloc >> 7
            order = np.lexsort((dstloc, w, k))
            ea = ea_list[c][order] if self.gat else None
            percore.append((gidx[order], dstloc[order], k[order], w[order], ea))
            np.add.at(counts[c], (k[order], w[order]), 1)
        S = np.maximum.reduce([np.ceil(counts[c] / P).astype(np.int64) for c in range(NCORES)])
        self.S = S                                   # [NCH, W] subtiles per group
        self.Smax = int(S.max()) if S.size else 0
        self.L = int(S.sum()) * P                    # padded edges per core
        # call layout per chunk
        self.calls = []                              # per k: list of call sizes
        self.sub2call = []                           # per subtile: (call_id, col)
        call_id = 0
        for k in range(cf.NCH):
            Lk = int(S[k].sum()) * P
            rem = Lk
            sizes = []
            while rem > 0:
                n = min(cf.CALL, rem)
                sizes.append(n)
                for col in range(n // P):
                    self.sub2call.append((call_id, col))
                call_id += 1
                rem -= n
            self.calls.append(sizes)
        self.n_calls = call_id
        self.nsub = self.L // P

        self.idx16, self.dstrel, self.ea, self.gridx16 = [], [], [], []
        for c in range(NCORES):
            gidx, dstloc, kk, ww, ea = percore[c]
            gi_parts, dr_parts, ea_parts, gr_parts = [], [], [], []
            for k in range(cf.NCH):
                for w in range(cf.W):
                    n = int(S[k, w]) * P
                    if n == 0:
                        continue
                    m = (kk == k) & (ww == w)
                    gi_parts.append(_pad_to(gidx[m] - k * cf.CHUNK, n, 0))
                    dr_parts.append(_pad_to((dstloc[m] - w * P).astype(np.float32), n, -1.0))
                    if self.gat:
                        ea_parts.append(_pad_to(ea[m].astype(np.float32), n, 0.0))
                        gr_parts.append(_pad_to(dstloc[m], n, 0))
            gi = np.concatenate(gi_parts) if gi_parts else np.zeros(0, np.int64)
            dr = np.concatenate(dr_parts) if dr_parts else np.zeros(0, np.float32)
            assert gi.shape[0] == self.L
            assert (gi >= 0).all() and (gi < 32768).all()

            def wrap_all(arr):
                off, wraps = 0, []
                for k in range(cf.NCH):
                    for n in self.calls[k]:
                        wraps.append(_wrap_idx_call(arr[off:off + n]))
                        off += n
                return (np.concatenate(wraps, axis=1) if wraps
                        else np.zeros((P, 0), np.int16))

            self.idx16.append(wrap_all(gi))
            self.dstrel.append(np.ascontiguousarray(dr.reshape(-1, P).T).astype(NPBF))
            if self.gat:
                self.ea.append(np.ascontiguousarray(
                    np.concatenate(ea_parts).reshape(-1, P).T).astype(NPBF))
                self.gridx16.append(wrap_all(np.concatenate(gr_parts)))

        # groups: per (k,w) with S>0: list of runs (sub0, nsub, call_id, col0)
        self.groups = []
        sub = 0
        for k in range(cf.NCH):
            for w in range(cf.W):
                s = int(S[k, w])
                if s == 0:
                    continue
                runs, i = [], sub
                while i < sub + s:
                    cid, col = self.sub2call[i]
                    n = 1
                    while i + n < sub + s and self.sub2call[i + n][0] == cid:
                        n += 1
                    runs.append((i, n, cid, col))
                    i += n
                self.groups.append((k, w, runs))
                sub += s
        assert sub == self.nsub


def preprocess(inputs, cfg):
    cf = cfg
    x = np.asarray(inputs["x"], np.float32)
    ei = np.asarray(inputs["edge_index"])
    ea = np.asarray(inputs["edge_attr"], np.float32).reshape(-1)
    batch = np.asarray(inputs["batch"]).astype(np.int64)
    src, dst = ei[0].astype(np.int64), ei[1].astype(np.int64)
    N, G = cf.N, cf.G

    deg_in = np.bincount(dst, minlength=N).astype(np.float32)
    deg_out = np.bincount(src, minlength=N).astype(np.float32)
    cnt = np.bincount(batch, minlength=G).astype(np.float32)

    core_of = lambda n: n // cf.N0
    m_in = [core_of(dst) == c for c in range(NCORES)]
    m_out = [core_of(src) == c for c in range(NCORES)]
    passes = {
        "in": EdgePass(cf, [src[m] for m in m_in], [dst[m] for m in m_in]),
        "out": EdgePass(cf, [dst[m] for m in m_out], [src[m] for m in m_out]),
    }
    gsrc, gdst, gea = [], [], []
    for c in range(NCORES):
        loop = np.arange(c * cf.N0, (c + 1) * cf.N0, dtype=np.int64)
        gsrc.append(np.concatenate([src[m_in[c]], loop]))
        gdst.append(np.concatenate([dst[m_in[c]], loop]))
        gea.append(np.concatenate([ea[m_in[c]], np.ones(cf.N0, np.float32)]))
    passes["gat"] = EdgePass(cf, gsrc, gdst, gea)

    iota = np.tile(np.arange(P, dtype=np.float32)[None, :], (P, 1))
    in_maps = []
    for c in range(NCORES):
        lo, hi = c * cf.N0, (c + 1) * cf.N0
        xl = np.zeros((cf.PADN0, D), NPBF)
        xl[:cf.N0] = x[lo:hi].astype(NPBF)
        nodecol = lambda v, pad: np.ascontiguousarray(
            _pad_to(v[lo:hi].astype(np.float32), cf.PADN0, pad).reshape(cf.W, P).T)
        m = {
            "x_local": xl,
            "deg_in_f": nodecol(deg_in, 0.0),
            "deg_out_f": nodecol(deg_out, 0.0),
            "batchloc": nodecol(batch.astype(np.float32), -1.0).astype(NPBF),
            "cnt_f": np.ascontiguousarray(_pad_to(cnt, cf.GPAD, 0.0).reshape(cf.GW, P).T),
            "iota_in": iota.astype(NPBF),
            "dir_w_in": np.asarray(inputs["dir_w_in"], np.float32).reshape(2 * D, D),
            "dir_b_in": np.asarray(inputs["dir_b_in"], np.float32),
            "dir_w_out": np.asarray(inputs["dir_w_out"], np.float32).reshape(2 * D, D),
            "dir_b_out": np.asarray(inputs["dir_b_out"], np.float32),
            "gat_wl": np.asarray(inputs["gat_wl"], np.float32),
            "gat_wr": np.asarray(inputs["gat_wr"], np.float32),
            "gat_we": np.asarray(inputs["gat_we"], np.float32),
            "gat_att": np.asarray(inputs["gat_att"], np.float32).reshape(1, D),
            "gat_b": np.asarray(inputs["gat_b"], np.float32).reshape(D, 1),
            "w1": np.asarray(inputs["w1"], np.float32),
            "b1": np.asarray(inputs["b1"], np.float32).reshape(1, HID),
            "w2": np.asarray(inputs["w2"], np.float32),
            "b2": np.asarray(inputs["b2"], np.float32).reshape(1, CLS),
        }
        for pname, ep in passes.items():
            m[f"idx_{pname}"] = ep.idx16[c]
            m[f"dstrel_{pname}"] = ep.dstrel[c]
            if ep.gat:
                m[f"ea_{pname}"] = ep.ea[c]
                m[f"gridx_{pname}"] = ep.gridx16[c]
        in_maps.append(m)
    return in_maps, passes


# ---------------------------------------------------------------- device side

def build_program(cfg, passes, stop_after=None, sim=False):
    cf = cfg
    nc = bacc.Bacc("TRN2", target_bir_lowering=False, debug=False, num_devices=NCORES,
                   num_swdge_queues=2)

    def inp(name, shape, dt=F32):
        return nc.dram_tensor(name, shape, dt, kind="ExternalInput")

    x_local = inp("x_local", [cf.PADN0, D], BF)
    deg_in_f = inp("deg_in_f", [P, cf.W])
    deg_out_f = inp("deg_out_f", [P, cf.W])
    batchloc_in = inp("batchloc", [P, cf.W], BF)
    cnt_f = inp("cnt_f", [P, cf.GW])
    iota_in = inp("iota_in", [P, P], BF)
    dir_w_in = inp("dir_w_in", [2 * D, D])
    dir_b_in = inp("dir_b_in", [2, D])
    dir_w_out = inp("dir_w_out", [2 * D, D])
    dir_b_out = inp("dir_b_out", [2, D])
    gat_wl = inp("gat_wl", [D, D])
    gat_wr = inp("gat_wr", [D, D])
    gat_we = inp("gat_we", [1, D])
    gat_att = inp("gat_att", [1, D])
    gat_b = inp("gat_b", [D, 1])
    w1_in = inp("w1", [2 * D, HID])
    b1_in = inp("b1", [1, HID])
    w2_in = inp("w2", [HID, CLS])
    b2_in = inp("b2", [1, CLS])
    pin = {}
    for pname, ep in passes.items():
        pin[pname] = {
            "idx": inp(f"idx_{pname}", [P, ep.L // 16], I16),
            "dstrel": inp(f"dstrel_{pname}", [P, ep.L // P], BF),
        }
        if ep.gat:
            pin[pname]["ea"] = inp(f"ea_{pname}", [P, ep.L // P], BF)
            pin[pname]["gridx"] = inp(f"gridx_{pname}", [P, ep.L // 16], I16)

    out_t = nc.dram_tensor("out", [cf.GPAD, CLS], F32, kind="ExternalOutput")

    loc = {n: nc.dram_tensor(f"{n}_loc", [cf.PADN0, D], BF, kind="Internal")
           for n in ("t1i", "t1o", "t2i", "t2o", "gl")}
    gr_loc = nc.dram_tensor("gr_loc", [cf.PADN0, D], BF, kind="Internal")
    tabs = {n: nc.dram_tensor(f"{n}_tab", [cf.TAB, D], BF, kind="Internal", addr_space="Shared")
            for n in ("t1i", "t1o", "t2i", "t2o", "gl")}
    pool_part = nc.dram_tensor("pool_part", [cf.GPAD, 2 * D], F32, kind="Internal")
    pool_red = nc.dram_tensor("pool_red", [cf.GPAD, 2 * D], F32, kind="Internal", addr_space="Shared")
    RG = [list(range(NCORES))]
    Smax = max(ep.Smax for ep in passes.values())
    RUNMAX = Smax

    with tile.TileContext(nc) as tc:
        with (
            tc.tile_pool(name="const", bufs=1) as cpool,
            tc.tile_pool(name="agg", bufs=1) as aggpool,
            tc.tile_pool(name="work", bufs=3) as pool,
            tc.tile_pool(name="work2", bufs=2) as pool2,
            tc.tile_pool(name="psum", bufs=2, space="PSUM") as psum,
        ):
            # ---- constants / weights
            iota = cpool.tile([P, P], BF)
            nc.scalar.dma_start(iota[:], iota_in[:, :])
            iota3 = iota[:].rearrange("p (o q) -> p o q", o=1)
            ones_row = cpool.tile([1, P], BF)
            nc.vector.memset(ones_row[:], 1.0)
            from concourse.masks import make_identity
            ident = cpool.tile([P, P], BF)
            make_identity(nc, ident[:])

            def pe_transpose(src_ap, out_tile_ap):
                tp = psum.tile([P, P], BF, space="PSUM", tag="tpsum", name="tp")
                nc.tensor.transpose(out=tp[:, :], in_=src_ap, identity=ident[:])
                nc.vector.tensor_copy(out=out_tile_ap, in_=tp[:, :])

            def load_w(src_ap, shape, name, dt=F32):
                t = cpool.tile(shape, dt, name=name)
                nc.scalar.dma_start(t[:], src_ap)
                return t

            def to_bf(t, shape, name):
                b = cpool.tile(shape, BF, name=name)
                nc.vector.tensor_copy(out=b[:], in_=t[:])
                return b

            Wi_f = [load_w(dir_w_in[i * D:(i + 1) * D, :], [D, D], f"cwif{i}") for i in range(2)]
            Wo_f = [load_w(dir_w_out[i * D:(i + 1) * D, :], [D, D], f"cwof{i}") for i in range(2)]
            for i in range(2):
                nc.vector.tensor_scalar_mul(out=Wi_f[i][:], in0=Wi_f[i][:], scalar1=1.0 - ALPHA)
                nc.vector.tensor_scalar_mul(out=Wo_f[i][:], in0=Wo_f[i][:], scalar1=ALPHA)
            Wi = [to_bf(Wi_f[i], [D, D], f"cwi{i}") for i in range(2)]
            Wo = [to_bf(Wo_f[i], [D, D], f"cwo{i}") for i in range(2)]
            bcomb = []
            for i in range(2):
                bi = load_w(dir_b_in[i:i + 1, :], [1, D], f"cbif{i}")
                bo = load_w(dir_b_out[i:i + 1, :], [1, D], f"cbof{i}")
                nc.vector.tensor_scalar_mul(out=bi[:], in0=bi[:], scalar1=1.0 - ALPHA)
                nc.vector.tensor_scalar_mul(out=bo[:], in0=bo[:], scalar1=ALPHA)
                nc.vector.tensor_tensor(out=bi[:], in0=bi[:], in1=bo[:], op=AOT.add)
                bcomb.append(to_bf(bi, [1, D], f"cbi{i}"))
            Wl = to_bf(load_w(gat_wl[:, :], [D, D], "cwlf"), [D, D], "cwl")
            Wr = to_bf(load_w(gat_wr[:, :], [D, D], "cwrf"), [D, D], "cwr")
            w1t = [to_bf(load_w(w1_in[i * D:(i + 1) * D, :], [D, HID], f"cw1f{i}"), [D, HID], f"cw1{i}")
                   for i in range(2)]
            b1r = load_w(b1_in[:, :], [1, HID], "cb1r")
            w2t = to_bf(load_w(w2_in[:, :], [HID, CLS], "cw2f"), [HID, CLS], "cw2t")
            b2r = to_bf(load_w(b2_in[:, :], [1, CLS], "cb2f"), [1, CLS], "cb2r")
            we_rep_f = cpool.tile([P, D], F32)
            nc.scalar.dma_start(we_rep_f[:], gat_we[0:1, :].to_broadcast([P, D]))
            we_rep = to_bf(we_rep_f, [P, D], "cwer")
            we3 = we_rep[:].rearrange("p (o q) -> p o q", o=1)
            att_rep_f = cpool.tile([P, D], F32)
            nc.scalar.dma_start(att_rep_f[:], gat_att[0:1, :].to_broadcast([P, D]))
            att_rep = to_bf(att_rep_f, [P, D], "catr")
            att3 = att_rep[:].rearrange("p (o q) -> p o q", o=1)
            gatb_col = to_bf(load_w(gat_b[:, :], [D, 1], "cgbcf"), [D, 1], "cgbc")
            batch_t = cpool.tile([P, cf.W], BF)
            nc.scalar.dma_start(batch_t[:], batchloc_in[:, :])

            psb = psum.tile([1, HID], F32, space="PSUM", tag="opsum")
            nc.tensor.matmul(out=psb[:, :], lhsT=gatb_col[:], rhs=w1t[1][:], start=True, stop=True)
            b1p = cpool.tile([1, HID], F32)
            nc.vector.tensor_tensor(out=b1p[:], in0=psb[:, :], in1=b1r[:], op=AOT.add)
            b1pb = to_bf(b1p, [1, HID], "cb1pb")

            def make_dinv(deg_dram, pfx):
                degt = cpool.tile([P, cf.W], F32, name=f"{pfx}degt")
                nc.scalar.dma_start(degt[:], deg_dram[:, :])
                m = cpool.tile([P, cf.W], F32, name=f"{pfx}m")
                nc.vector.tensor_scalar_max(out=m[:], in0=degt[:], scalar1=1.0)
                nc.scalar.activation(m[:], m[:], AFT.Sqrt)
                r = cpool.tile([P, cf.W], F32, name=f"{pfx}r")
                nc.vector.reciprocal(out=r[:], in_=m[:])
                nc.vector.tensor_scalar(out=m[:], in0=degt[:], scalar1=0.0,
                                        scalar2=None, op0=AOT.is_gt)
                nc.vector.tensor_tensor(out=r[:], in0=r[:], in1=m[:], op=AOT.mult)
                return r

            dinv_i = make_dinv(deg_in_f, "di_")
            dinv_o = make_dinv(deg_out_f, "do_")
            inv_cnt = cpool.tile([P, cf.GW], F32)
            nc.scalar.dma_start(inv_cnt[:], cnt_f[:, :])
            nc.vector.tensor_scalar_max(out=inv_cnt[:], in0=inv_cnt[:], scalar1=1.0)
            nc.vector.reciprocal(out=inv_cnt[:], in_=inv_cnt[:])

            # ---- phase 1: scaled x tables (t1i/t1o), gl, gr
            for dinv, dst, tag in ((dinv_i, loc["t1i"], "aggA"), (dinv_o, loc["t1o"], "aggB")):
                sc = aggpool.tile([P, cf.W, D], BF, tag=tag)
                nc.scalar.dma_start(sc[:, :, :], x_local[:, :].rearrange("(w p) d -> p w d", p=P))
                nc.vector.tensor_tensor(out=sc[:, :, :], in0=sc[:, :, :],
                                        in1=dinv[:].to_broadcast([P, cf.W, D]), op=AOT.mult)
                nc.sync.dma_start(dst[:, :].rearrange("(w p) d -> p w d", p=P), sc[:, :, :])
            for r in range(cf.W):
                xt = pool.tile([P, D], BF, tag="rowa")
                nc.scalar.dma_start(xt[:], x_local[r * P:(r + 1) * P, :])
                xT = pool.tile([P, P], BF, tag="rowb")
                pe_transpose(xt[:], xT[:])
                for Wmat, dst, tg in ((Wl, loc["gl"], "rowc"), (Wr, gr_loc, "rowd")):
                    ps = psum.tile([P, D], F32, space="PSUM", tag="opsum")
                    nc.tensor.matmul(out=ps[:, :], lhsT=xT[:], rhs=Wmat[:], start=True, stop=True)
                    ot = pool.tile([P, D], BF, tag=tg)
                    nc.vector.tensor_copy(out=ot[:], in_=ps[:, :])
                    nc.scalar.dma_start(dst[r * P:(r + 1) * P, :], ot[:])
            for n in ("t1i", "t1o", "gl"):
                if sim:
                    nc.sync.dma_start(tabs[n][0:cf.PADN0, :], loc[n][:, :])
                else:
                    nc.gpsimd.collective_compute("AllGather", AOT.bypass, replica_groups=RG,
                                                 ins=[loc[n][:, :]], outs=[tabs[n][:, :]])


            # ---- edge pass emitter
            def edge_pass(ep, pi, table, agg_tag, gat=False, sfx=""):
                ecols = 132 if gat else D
                agg = aggpool.tile([P, cf.W, ecols], F32, tag=agg_tag)
                nc.vector.memset(agg[:, :, :], 0.0)
                dstrel = pool2.tile([P, max(ep.L // P, 1)], BF, tag="dr", bufs=1)
                nc.scalar.dma_start(dstrel[:, :], pi["dstrel"][:, :])
                if gat:
                    eat = pool2.tile([P, max(ep.L // P, 1)], BF, tag="ea", bufs=1)
                    nc.scalar.dma_start(eat[:, :], pi["ea"][:, :])
                call_meta = []
                coff = 0
                for k in range(cf.NCH):
                    for n in ep.calls[k]:
                        call_meta.append((k, n, coff))
                        coff += n
                gtiles = {}

                def get_call(cid):
                    if cid in gtiles:
                        return gtiles[cid]
                    k, n, off = call_meta[cid]
                    it = pool.tile([P, cf.CALL // 16], I16, tag="idxt" + sfx)
                    nc.sync.dma_start(it[:, :n // 16], pi["idx"][:, off // 16:(off + n) // 16])
                    g = pool.tile([P, cf.CALL // P, D], BF, tag="gt" + sfx)
                    tab_view = table[k * cf.CHUNK:min((k + 1) * cf.CHUNK, cf.TAB), :]
                    nc.gpsimd.dma_gather(g[:, :n // P, :], tab_view, it[:, :n // 16],
                                         num_idxs=n, num_idxs_reg=n, elem_size=D,
                                         queue_num=cid % 2)
                    g2 = None
                    if gat:
                        it2 = pool.tile([P, cf.CALL // 16], I16, tag="idxt2" + sfx)
                        nc.sync.dma_start(it2[:, :n // 16], pi["gridx"][:, off // 16:(off + n) // 16])
                        g2 = pool.tile([P, cf.CALL // P, D], BF, tag="gt" + sfx)
                        nc.gpsimd.dma_gather(g2[:, :n // P, :], gr_loc[:, :], it2[:, :n // 16],
                                             num_idxs=n, num_idxs_reg=n, elem_size=D,
                                             queue_num=(cid + 1) % 2)
                    gtiles[cid] = (g, g2)
                    for old in [c for c in gtiles if c < cid - 1]:
                        del gtiles[old]
                    return gtiles[cid]

                for k, w, runs in ep.groups:
                    s0 = runs[0][0]
                    S = sum(r[1] for r in runs)
                    nmm = S
                    oh = pool.tile([P, Smax, P], BF, tag="oh" + sfx)
                    nc.vector.tensor_tensor(
                        out=oh[:, :S, :],
                        in0=dstrel[:, s0:s0 + S].rearrange("p (s o) -> p s o", o=1).to_broadcast([P, S, P]),
                        in1=iota3.to_broadcast([P, S, P]),
                        op=AOT.is_equal)
                    ps = psum.tile([P, ecols], F32, space="PSUM", tag="epsum")
                    mm = 0
                    for (sub0, nsub, cid, col0) in runs:
                        g, g2 = get_call(cid)
                        if gat:
                            rhs = pool.tile([P, RUNMAX, 132], BF, tag="gatrhs")
                            gl_s = g[:, col0:col0 + nsub, :]
                            tmp = pool2.tile([P, RUNMAX, D], BF, tag="gtmp")
                            u = pool2.tile([P, RUNMAX, D], BF, tag="gtmp2")
                            tmps = tmp[:, :nsub, :]
                            us = u[:, :nsub, :]
                            nc.vector.tensor_tensor(out=tmps, in0=gl_s,
                                                    in1=g2[:, col0:col0 + nsub, :], op=AOT.add)
                            nc.vector.tensor_tensor(
                                out=us,
                                in0=eat[:, sub0:sub0 + nsub].rearrange("p (s o) -> p s o", o=1).to_broadcast([P, nsub, D]),
                                in1=we3.to_broadcast([P, nsub, D]), op=AOT.mult)
                            nc.vector.tensor_tensor(out=tmps, in0=tmps, in1=us, op=AOT.add)
                            nc.vector.tensor_scalar_mul(out=us, in0=tmps, scalar1=NEG)
                            nc.vector.tensor_tensor(out=tmps, in0=tmps, in1=us, op=AOT.max)
                            nc.vector.tensor_tensor(out=us, in0=tmps,
                                                    in1=att3.to_broadcast([P, nsub, D]), op=AOT.mult)
                            lg = pool.tile([P, RUNMAX, H], F32, tag="lg")
                            nc.vector.tensor_reduce(
                                out=lg[:, :nsub, :],
                                in_=u[:, :nsub, :].rearrange("p s (h c) -> p s h c", h=H),
                                axis=AXX.X, op=AOT.add)
                            nc.scalar.activation(rhs[:, :nsub, D:132], lg[:, :nsub, :], AFT.Exp)
                            nc.vector.tensor_tensor(
                                out=rhs[:, :nsub, 0:D].rearrange("p s (h c) -> p s h c", c=C),
                                in0=gl_s.rearrange("p s (h c) -> p s h c", c=C),
                                in1=rhs[:, :nsub, D:132].rearrange("p s (h o) -> p s h o", o=1).to_broadcast([P, nsub, H, C]),
                                op=AOT.mult)
                            for j in range(nsub):
                                nc.tensor.matmul(out=ps[:, :], lhsT=oh[:, sub0 - s0 + j, :],
                                                 rhs=rhs[:, j, :], start=(mm == 0), stop=(mm == nmm - 1),
                                                 skip_group_check=True)
                                mm += 1
                        else:
                            for j in range(nsub):
                                nc.tensor.matmul(out=ps[:, :], lhsT=oh[:, sub0 - s0 + j, :],
                                                 rhs=g[:, col0 + j, :], start=(mm == 0), stop=(mm == nmm - 1),
                                                 skip_group_check=True)
                                mm += 1
                    nc.vector.tensor_tensor(out=agg[:, w, :], in0=agg[:, w, :], in1=ps[:, :], op=AOT.add)
                return agg

            # ---- GCN layers
            def gcn_layer(tin, tout, layer, per_row):
                agg_i = edge_pass(passes["in"], pin["in"], tin, "aggA", sfx="i")
                agg_o = edge_pass(passes["out"], pin["out"], tout, "aggB", sfx="o")
                li = layer - 1
                for r in range(cf.W):
                    si = pool.tile([P, D], BF, tag="rowa")
                    nc.vector.tensor_tensor(out=si[:], in0=agg_i[:, r, 0:D],
                                            in1=dinv_i[:, r:r + 1].to_broadcast([P, D]), op=AOT.mult)
                    so = pool.tile([P, D], BF, tag="rowb")
                    nc.vector.tensor_tensor(out=so[:], in0=agg_o[:, r, 0:D],
                                            in1=dinv_o[:, r:r + 1].to_broadcast([P, D]), op=AOT.mult)
                    siT = pool.tile([P, P], BF, tag="rowc")
                    pe_transpose(si[:], siT[:])
                    soT = pool.tile([P, P], BF, tag="rowd")
                    pe_transpose(so[:], soT[:])
                    ps = psum.tile([P, D], F32, space="PSUM", tag="opsum")
                    nc.tensor.matmul(out=ps[:, :], lhsT=siT[:], rhs=Wi[li][:], start=True, stop=False)
                    nc.tensor.matmul(out=ps[:, :], lhsT=soT[:], rhs=Wo[li][:], start=False, stop=False)
                    nc.tensor.matmul(out=ps[:, :], lhsT=ones_row[:], rhs=bcomb[li][:], start=False, stop=True)
                    ht = pool.tile([P, D], BF, tag="rowe")
                    nc.scalar.activation(ht[:], ps[:, :], AFT.Relu)
                    per_row(r, ht)

            def pool_onehot(r, gw):
                bl = pool.tile([P, 1], BF, tag="bl")
                nc.vector.tensor_scalar_add(out=bl[:], in0=batch_t[:, r:r + 1], scalar1=float(-P * gw))
                po = pool.tile([P, P], BF, tag="po")
                nc.vector.tensor_tensor(out=po[:], in0=bl[:].to_broadcast([P, P]),
                                        in1=iota[:], op=AOT.is_equal)
                return po

            # layer 1 -> t2 tables
            def l1_row(r, ht):
                for dinv, dst, tg in ((dinv_i, loc["t2i"], "rowf"), (dinv_o, loc["t2o"], "rowg")):
                    sc = pool.tile([P, D], BF, tag=tg)
                    nc.vector.tensor_tensor(out=sc[:], in0=ht[:],
                                            in1=dinv[:, r:r + 1].to_broadcast([P, D]), op=AOT.mult)
                    nc.scalar.dma_start(dst[r * P:(r + 1) * P, :], sc[:])

            gcn_layer(tabs["t1i"], tabs["t1o"], 1, l1_row)
            for n in ("t2i", "t2o"):
                if sim:
                    nc.sync.dma_start(tabs[n][0:cf.PADN0, :], loc[n][:, :])
                else:
                    nc.gpsimd.collective_compute("AllGather", AOT.bypass, replica_groups=RG,
                                                 ins=[loc[n][:, :]], outs=[tabs[n][:, :]])

            # layer 2 -> pool x1
            pool_ps1 = [psum.tile([P, D], F32, space="PSUM", tag="poolps", name=f"poolps1_{gw}") for gw in range(cf.GW)]

            def l2_row(r, ht):
                for gw in range(cf.GW):
                    po = pool_onehot(r, gw)
                    nc.tensor.matmul(out=pool_ps1[gw][:, :], lhsT=po[:], rhs=ht[:],
                                     start=(r == 0), stop=(r == cf.W - 1), skip_group_check=True)

            gcn_layer(tabs["t2i"], tabs["t2o"], 2, l2_row)
            x1part = cpool.tile([P, cf.GW, D], F32)
            for gw in range(cf.GW):
                nc.vector.tensor_copy(out=x1part[:, gw, :], in_=pool_ps1[gw][:, :])

            numden = edge_pass(passes["gat"], pin["gat"], tabs["gl"], "aggA", gat=True, sfx="i")
            pool_ps2 = [psum.tile([P, D], F32, space="PSUM", tag="poolps", name=f"poolps2_{gw}") for gw in range(cf.GW)]
            for r in range(cf.W):
                den = pool.tile([P, H], F32, tag="den")
                nc.vector.tensor_scalar_max(out=den[:], in0=numden[:, r, D:132], scalar1=1e-30)
                nc.vector.reciprocal(out=den[:], in_=den[:])
                o2 = pool.tile([P, D], BF, tag="rowa")
                nc.vector.tensor_tensor(
                    out=o2[:].rearrange("p (h c) -> p h c", c=C),
                    in0=numden[:, r, 0:D].rearrange("p (h c) -> p h c", c=C),
                    in1=den[:].rearrange("p (h o) -> p h o", o=1).to_broadcast([P, H, C]),
                    op=AOT.mult)
                for gw in range(cf.GW):
                    po = pool_onehot(r, gw)
                    nc.tensor.matmul(out=pool_ps2[gw][:, :], lhsT=po[:], rhs=o2[:],
                                     start=(r == 0), stop=(r == cf.W - 1), skip_group_check=True)
            x2part = cpool.tile([P, cf.GW, D], F32)
            for gw in range(cf.GW):
                nc.vector.tensor_copy(out=x2part[:, gw, :], in_=pool_ps2[gw][:, :])

            # ---- pooled AllReduce + head
            nc.sync.dma_start(pool_part[:, 0:D].rearrange("(g p) d -> p g d", p=P), x1part[:, :, :])
            nc.sync.dma_start(pool_part[:, D:2 * D].rearrange("(g p) d -> p g d", p=P), x2part[:, :, :])
            if sim:
                nc.sync.dma_start(pool_red[:, :], pool_part[:, :])
            else:
                nc.gpsimd.collective_compute("AllReduce", AOT.add, replica_groups=RG,
                                             ins=[pool_part[:, :]], outs=[pool_red[:, :]])
            for gw in range(cf.GW):
                z = pool.tile([P, 2 * D], F32, tag="z")
                nc.scalar.dma_start(z[:], pool_red[gw * P:(gw + 1) * P, :])
                nc.vector.tensor_tensor(
                    out=z[:].rearrange("p (a d) -> p a d", a=2),
                    in0=z[:].rearrange("p (a d) -> p a d", a=2),
                    in1=inv_cnt[:, gw:gw + 1].rearrange("p (o q) -> p o q", q=1).to_broadcast([P, 2, D]),
                    op=AOT.mult)
                zb = pool.tile([P, 2 * D], BF, tag="zb")
                nc.vector.tensor_copy(out=zb[:], in_=z[:])
                ps = psum.tile([P, HID], F32, space="PSUM", tag="opsum")
                for i in range(2):
                    zT = pool.tile([P, P], BF, tag="rowb")
                    pe_transpose(zb[:, i * D:(i + 1) * D], zT[:])
                    nc.tensor.matmul(out=ps[:, :], lhsT=zT[:], rhs=w1t[i][:], start=(i == 0), stop=False)
                nc.tensor.matmul(out=ps[:, :], lhsT=ones_row[:], rhs=b1pb[:], start=False, stop=True)
                zh = pool.tile([P, HID], BF, tag="rowc")
                nc.scalar.activation(zh[:], ps[:, :], AFT.Relu)
                zhT = pool.tile([P, P], BF, tag="rowd")
                pe_transpose(zh[:], zhT[:])
                ps2 = psum.tile([P, CLS], F32, space="PSUM", tag="opsum")
                nc.tensor.matmul(out=ps2[:, :], lhsT=zhT[:], rhs=w2t[:], start=True, stop=False)
                nc.tensor.matmul(out=ps2[:, :], lhsT=ones_row[:], rhs=b2r[:], start=False, stop=True)
                og = pool.tile([P, CLS], F32, tag="roweo")
                nc.vector.tensor_copy(out=og[:], in_=ps2[:, :])
                nc.scalar.dma_start(out_t[gw * P:(gw + 1) * P, :], og[:])

    nc.compile()
    return nc


# ---------------------------------------------------------------- runner

import time
import jax
from jax.sharding import Mesh, PartitionSpec
from jax.experimental.shard_map import shard_map
from concourse.bass2jax import _bass_exec_p, install_neuronx_cc_hook, partition_id_tensor


class BassRunner:
    def __init__(self, nc, n_cores=8):
        install_neuronx_cc_hook()
        self.nc = nc
        self.n_cores = n_cores
        self.partition_name = nc.partition_id_tensor.name if nc.partition_id_tensor else None
        self.dbg_name = nc.dbg_addr.name if nc.dbg_addr is not None else None
        in_names, out_names, out_avals, zero_outs = [], [], [], []
        for alloc in nc.m.functions[0].allocations:
            if not isinstance(alloc, mybir.MemoryLocationSet):
                continue
            name = alloc.memorylocations[0].name
            if alloc.kind == "ExternalInput":
                if name != self.partition_name:
                    in_names.append(name)
            elif alloc.kind == "ExternalOutput":
                shape = tuple(alloc.tensor_shape)
                dtype = mybir.dt.np(alloc.dtype)
                out_names.append(name)
                out_avals.append(jax.core.ShapedArray(shape, dtype))
                zero_outs.append(np.zeros(shape, dtype))
        self.in_names, self.out_names = in_names, out_names
        self.out_avals, self.zero_outs = out_avals, zero_outs
        n_params, n_outs = len(in_names), len(out_avals)
        all_in_names = list(in_names) + list(out_names)
        if self.partition_name is not None:
            all_in_names.append(self.partition_name)
        donate = tuple(range(n_params, n_params + n_outs))

        def _body(*args):
            operands = list(args)
            if self.partition_name is not None:
                operands.append(partition_id_tensor())
            outs = _bass_exec_p.bind(
                *operands, out_avals=tuple(out_avals), in_names=tuple(all_in_names),
                out_names=tuple(out_names), lowering_input_output_aliases=(),
                sim_require_finite=True, sim_require_nnan=True, nc=nc)
            return tuple(outs)

        devices = jax.devices()[:n_cores]
        mesh = Mesh(np.asarray(devices), ("core",))
        in_specs = (PartitionSpec("core"),) * (n_params + n_outs)
        out_specs = (PartitionSpec("core"),) * n_outs
        self.fn = jax.jit(
            shard_map(_body, mesh=mesh, in_specs=in_specs, out_specs=out_specs, check_rep=False),
            donate_argnums=donate, keep_unused=True)

    def run(self, in_maps, repeats=1):
        if self.dbg_name is not None:
            in_maps = [{**m, self.dbg_name: np.zeros((1, 2), np.uint32)} for m in in_maps]
        per_core = [[np.asarray(m[name]) for name in self.in_names] for m in in_maps]
        concat_in = [np.concatenate([per_core[c][i] for c in range(self.n_cores)], axis=0)
                     for i in range(len(self.in_names))]
        import jax as _jax
        from jax.sharding import NamedSharding as _NS, PartitionSpec as _PS
        from jax.sharding import Mesh as _Mesh
        _mesh = _Mesh(np.asarray(_jax.devices()[:self.n_cores]), ("core",))
        concat_in = [_jax.device_put(a, _NS(_mesh, _PS("core"))) for a in concat_in]
        concat_in = _jax.block_until_ready(concat_in)
        from jax.sharding import NamedSharding
        mesh = self.fn_mesh if hasattr(self, "fn_mesh") else None
        times, first_out = [], None
        for _ in range(repeats):
            concat_zeros = [np.zeros((self.n_cores * z.shape[0], *z.shape[1:]), z.dtype)
                            for z in self.zero_outs]
            t0 = time.perf_counter()
            out_arrs = jax.block_until_ready(self.fn(*concat_in, *concat_zeros))
            times.append(time.perf_counter() - t0)
            if first_out is None:
                first_out = [
                    {name: np.asarray(out_arrs[i]).reshape(self.n_cores, *self.out_avals[i].shape)[c]
                     for i, name in enumerate(self.out_names)}
                    for c in range(self.n_cores)]
        return first_out, times


# ---------------------------------------------------------------- entry point

_N, _E, _G = 100000, 1000000, 250


def kernel(**inputs):
    cfg = Cfg(_N, _E, _G)
    in_maps, passes = preprocess(inputs, cfg)
    nc = build_program(cfg, passes)
    r = BassRunner(nc, NCORES)
    outs, _ = r.run(in_maps, repeats=1)
    return np.ascontiguousarray(outs[0]["out"][:_G].astype(np.float32))

